# revision 9
# baseline (speedup 1.0000x reference)
"""MoE layer (8 experts, top-2) on 8 TRN2 NeuronCores.

Strategy (expert-parallel, sparse dispatch):
  - Every core receives the full hidden_states plus ONE expert's weights
    (host-sharded across cores) and replicated router weights.
  - On device, each core computes the router (fp32 matmuls), softmax-free
    top-2 combine weights, and the load-balance loss for ALL 8192 tokens.
  - Each core compacts the tokens routed to ITS expert via a matmul-based
    prefix-sum over the selection mask, then scatters packed rows
    [token_id, combine_w, x(bf16)] into a capacity-bounded staging buffer
    with one indirect DMA per 128-token tile (OOB slots dropped).
  - The expert MLP (gelu(x@w1+b1)@w2+b2, bf16 matmuls, fp32 accumulate)
    runs over CAP staged slots; outputs are scaled by the combine weight
    and scattered back to a [8192,1024] bf16 buffer by token id.
  - A ReduceScatter(+) over the 8 cores combines the per-expert partial
    outputs; each core emits its 1024-token shard, host concatenates.
"""
import os

import numpy as np

# ---- problem constants (hardcoded per contest contract) ----
B, S, D, F, E = 4, 2048, 1024, 4096, 8
T = B * S                    # 8192 tokens
NT = T // 128                # 64 token tiles
NCORES = 8
Z_LOSS_COEF = 0.001

SPARSE = os.environ.get("MOE_SPARSE", "1") == "1"
CAP = int(os.environ.get("MOE_CAP", "3072"))   # per-expert token capacity
TT = 512                      # expert-phase token tile
ROWW = 2 + D // 2             # packed staging row in f32 words: id, c, x(bf16)
BIGPOS = 100000.0

_STATE = {}


def _build():
    import concourse.bass as bass
    import concourse.bacc as bacc
    import concourse.mybir as mybir
    import concourse.tile as tile

    dt = mybir.dt
    Act = mybir.ActivationFunctionType
    Alu = mybir.AluOpType

    nc = bacc.Bacc("TRN2", target_bir_lowering=False, debug=False,
                   num_devices=NCORES)

    # ---- I/O ----
    hidden = nc.dram_tensor("hidden", [T, D], dt.float32, kind="ExternalInput")
    rw_in = nc.dram_tensor("rw", [128, 64], dt.float32, kind="ExternalInput")
    w1_in = nc.dram_tensor("w1", [D, F], dt.float32, kind="ExternalInput")
    b1_in = nc.dram_tensor("b1p", [128, F // 128], dt.float32, kind="ExternalInput")
    w2_in = nc.dram_tensor("w2", [F, D], dt.float32, kind="ExternalInput")
    b2_in = nc.dram_tensor("b2p", [128, D // 128], dt.float32, kind="ExternalInput")
    esel_in = nc.dram_tensor("esel", [128, E], dt.float32, kind="ExternalInput")
    ids_in = nc.dram_tensor("ids", [128, NT], dt.float32, kind="ExternalInput")
    tri_in = nc.dram_tensor("tri", [128, 128], dt.float32, kind="ExternalInput")
    ones_in = nc.dram_tensor("ones", [128, 128], dt.float32, kind="ExternalInput")
    idf_in = nc.dram_tensor("idf", [128, 128], dt.float32, kind="ExternalInput")
    idb_in = nc.dram_tensor("idb", [128, 128], dt.bfloat16, kind="ExternalInput")

    out_ext = nc.dram_tensor("out", [T // NCORES, D], dt.float32, kind="ExternalOutput")
    lb_ext = nc.dram_tensor("lb", [1, 1], dt.float32, kind="ExternalOutput")

    # ---- internal DRAM ----
    xbf_dram = nc.dram_tensor("xbf_dram", [T, D], dt.bfloat16)
    w2bf_dram = nc.dram_tensor("w2bf_dram", [F, D], dt.bfloat16)
    out_buf = nc.dram_tensor("out_buf", [T, D], dt.bfloat16)
    rs_out = nc.dram_tensor("rs_out", [T // NCORES, D], dt.bfloat16)
    if SPARSE:
        staging = nc.dram_tensor("staging", [CAP, ROWW], dt.float32)

    n_exp_tiles = (CAP if SPARSE else T) // TT

    with tile.TileContext(nc) as tc:
        with tc.tile_pool(name="const", bufs=1) as cp:
            # ---- persistent tiles ----
            rw = cp.tile([128, 64], dt.float32, tag="rw")
            esel = cp.tile([128, E], dt.float32, tag="esel")
            ids = cp.tile([128, NT], dt.float32, tag="ids")
            tri = cp.tile([128, 128], dt.float32, tag="tri")
            ones = cp.tile([128, 128], dt.float32, tag="ones")
            idf = cp.tile([128, 128], dt.float32, tag="idf")
            idb = cp.tile([128, 128], dt.bfloat16, tag="idb")
            b1p = cp.tile([128, F // 128], dt.float32, tag="b1p")
            b2p = cp.tile([128, D // 128], dt.float32, tag="b2p")
            for t_, src in ((rw, rw_in), (esel, esel_in), (ids, ids_in),
                            (tri, tri_in), (ones, ones_in), (idf, idf_in),
                            (idb, idb_in), (b1p, b1_in), (b2p, b2_in)):
                nc.sync.dma_start(t_[:], src.ap())

            c_all = cp.tile([128, NT], dt.float32, tag="c_all")
            acc = cp.tile([128, E], dt.float32, tag="acc")
            nc.vector.memset(acc[:], 0.0)

            w1bf = cp.tile([128, D // 128 * F], dt.bfloat16, tag="w1bf")
            hT = cp.tile([128, F // 128 * TT], dt.bfloat16, tag="hT")
            pos_int = cp.tile([128, NT], dt.int32, tag="pos_int")

            # ---- phase W: weight conversion ----
            with tc.tile_pool(name="wconv", bufs=2) as wp:
                for c in range(D // 128):
                    st = wp.tile([128, F], dt.float32, tag="w1st")
                    nc.sync.dma_start(st[:], w1_in.ap()[c * 128:(c + 1) * 128, :])
                    nc.scalar.copy(w1bf[:, c * F:(c + 1) * F], st[:])
                for c in range(F // 128):
                    st2 = wp.tile([128, D], dt.float32, tag="w2st")
                    nc.sync.dma_start(st2[:], w2_in.ap()[c * 128:(c + 1) * 128, :])
                    st2b = wp.tile([128, D], dt.bfloat16, tag="w2stb")
                    nc.vector.tensor_copy(st2b[:], st2[:])
                    nc.sync.dma_start(w2bf_dram.ap()[c * 128:(c + 1) * 128, :], st2b[:])

            # zero-fill the output scatter buffer (only scattered rows get data)
            zrow = cp.tile([128, D], dt.bfloat16, tag="zrow")
            nc.vector.memset(zrow[:], 0.0)
            if SPARSE:
                for k in range(NT):
                    nc.sync.dma_start(out_buf.ap()[k * 128:(k + 1) * 128, :], zrow[:])

            # ---- phase A: router over all tokens ----
            with (
                tc.tile_pool(name="pa_sb", bufs=3) as pa,
                tc.tile_pool(name="pa_small", bufs=4) as pas,
                tc.tile_pool(name="pa_psT", bufs=2, space="PSUM") as paT,
                tc.tile_pool(name="pa_psL", bufs=2, space="PSUM") as paL,
            ):
                for k in range(NT):
                    x = pa.tile([128, D], dt.float32, tag="x")
                    nc.sync.dma_start(x[:], hidden.ap()[k * 128:(k + 1) * 128, :])
                    xb = pa.tile([128, D], dt.bfloat16, tag="xb")
                    nc.scalar.copy(xb[:], x[:])
                    nc.sync.dma_start(xbf_dram.ap()[k * 128:(k + 1) * 128, :], xb[:])

                    xt = pa.tile([128, D], dt.float32, tag="xt")
                    for h in range(2):
                        pt = paT.tile([128, 512], dt.float32, tag="pt")
                        for c4 in range(4):
                            c = 4 * h + c4
                            nc.tensor.transpose(
                                pt[:, c4 * 128:(c4 + 1) * 128],
                                x[:, c * 128:(c + 1) * 128], idf[:])
                        nc.vector.tensor_copy(xt[:, h * 512:(h + 1) * 512], pt[:])

                    pl = paL.tile([128, E], dt.float32, tag="pl")
                    for c in range(8):
                        nc.tensor.matmul(
                            pl[:], lhsT=xt[:, c * 128:(c + 1) * 128],
                            rhs=rw[:, c * 8:(c + 1) * 8],
                            start=(c == 0), stop=(c == 7))

                    ex = pas.tile([128, E], dt.float32, tag="ex")
                    ssum = pas.tile([128, 1], dt.float32, tag="ssum")
                    nc.scalar.activation(ex[:], pl[:], Act.Exp, accum_out=ssum[:])
                    srec = pas.tile([128, 1], dt.float32, tag="srec")
                    nc.vector.reciprocal(srec[:], ssum[:])
                    m8 = pas.tile([128, 8], dt.float32, tag="m8")
                    nc.vector.max(m8[:], ex[:])
                    dn = pas.tile([128, 1], dt.float32, tag="dn")
                    nc.vector.tensor_tensor(
                        out=dn[:], in0=m8[:, 0:1], in1=m8[:, 1:2], op=Alu.add)
                    rd = pas.tile([128, 1], dt.float32, tag="rd")
                    nc.vector.reciprocal(rd[:], dn[:])
                    cu = pas.tile([128, E], dt.float32, tag="cu")
                    nc.vector.scalar_tensor_tensor(
                        out=cu[:], in0=ex[:], scalar=m8[:, 1:2], in1=ex[:],
                        op0=Alu.is_ge, op1=Alu.mult)
                    scr = pas.tile([128, E], dt.float32, tag="scr")
                    nc.vector.scalar_tensor_tensor(
                        out=scr[:], in0=cu[:], scalar=rd[:], in1=esel[:],
                        op0=Alu.mult, op1=Alu.mult,
                        accum_out=c_all[:, k:k + 1])
                    nc.vector.scalar_tensor_tensor(
                        out=acc[:], in0=ex[:], scalar=srec[:], in1=acc[:],
                        op0=Alu.mult, op1=Alu.add)

            # ---- load-balance loss ----
            with tc.tile_pool(name="lb_sb", bufs=1) as lp:
                loadr = lp.tile([1, E], dt.float32, tag="loadr")
                nc.gpsimd.tensor_reduce(
                    out=loadr[:], in_=acc[:], axis=mybir.AxisListType.C, op=Alu.add)
                nc.vector.tensor_scalar(
                    out=loadr[:], in0=loadr[:], scalar1=1.0 / T,
                    scalar2=-1.0 / E, op0=Alu.mult, op1=Alu.add)
                sq = lp.tile([1, E], dt.float32, tag="sq")
                nc.scalar.square(sq[:], loadr[:])
                lbv = lp.tile([1, 1], dt.float32, tag="lbv")
                nc.vector.tensor_reduce(
                    out=lbv[:], in_=sq[:], axis=mybir.AxisListType.X, op=Alu.add)
                nc.vector.tensor_scalar(
                    out=lbv[:], in0=lbv[:], scalar1=Z_LOSS_COEF / E,
                    scalar2=None, op0=Alu.mult)
                nc.sync.dma_start(lb_ext.ap(), lbv[:])

            if SPARSE:
                # ---- phase B: compaction (positions via matmul prefix-sum) ----
                with (
                    tc.tile_pool(name="pb_sb", bufs=1) as pb,
                    tc.tile_pool(name="pb_ps", bufs=1, space="PSUM") as pbp,
                ):
                    mm_ = pb.tile([128, NT], dt.float32, tag="mm")
                    nc.vector.tensor_scalar(
                        out=mm_[:], in0=c_all[:], scalar1=0.0, scalar2=None,
                        op0=Alu.is_gt)
                    cs_ps = pbp.tile([NT, 1], dt.float32, tag="cs_ps")
                    nc.tensor.matmul(cs_ps[:], lhsT=mm_[:], rhs=ones[:, 0:1],
                                     start=True, stop=True)
                    cs = pb.tile([NT, 1], dt.float32, tag="cs")
                    nc.vector.tensor_copy(cs[:], cs_ps[:])
                    tric = pb.tile([NT, NT], dt.float32, tag="tric")
                    nc.vector.tensor_scalar(
                        out=tric[:], in0=tri[:NT, :NT], scalar1=cs[:, 0:1],
                        scalar2=None, op0=Alu.mult)
                    pos_ps = pbp.tile([128, NT], dt.float32, tag="pos_ps")
                    nc.tensor.matmul(pos_ps[:], lhsT=ones[:NT, :],
                                     rhs=tric[:], start=True, stop=False)
                    nc.tensor.matmul(pos_ps[:], lhsT=tri[:], rhs=mm_[:],
                                     start=False, stop=True)
                    posf = pb.tile([128, NT], dt.float32, tag="posf")
                    # pos + (1-mask)*BIG  ==  (mask*-BIG + pos) + BIG
                    nc.vector.scalar_tensor_tensor(
                        out=posf[:], in0=mm_[:], scalar=-BIGPOS, in1=pos_ps[:],
                        op0=Alu.mult, op1=Alu.add)
                    nc.vector.tensor_scalar(
                        out=posf[:], in0=posf[:], scalar1=BIGPOS, scalar2=None,
                        op0=Alu.add)
                    nc.vector.tensor_copy(pos_int[:], posf[:])

                # ---- phase C: pack + scatter tokens to staging ----
                with tc.tile_pool(name="pc_sb", bufs=3) as pc:
                    padrow = pc.tile([128, ROWW], dt.float32, tag="padrow")
                    nc.vector.memset(padrow[:], 0.0)
                    nc.vector.memset(padrow[:, 0:1], float(T))
                    for g in range(CAP // 128):
                        nc.sync.dma_start(
                            staging.ap()[g * 128:(g + 1) * 128, :], padrow[:])
                    for k in range(NT):
                        pk = pc.tile([128, ROWW], dt.float32, tag="pk")
                        nc.sync.dma_start(
                            pk[:, 2:ROWW].bitcast(dt.bfloat16),
                            xbf_dram.ap()[k * 128:(k + 1) * 128, :])
                        nc.vector.tensor_copy(pk[:, 0:1], ids[:, k:k + 1])
                        nc.vector.tensor_copy(pk[:, 1:2], c_all[:, k:k + 1])
                        nc.gpsimd.indirect_dma_start(
                            out=staging.ap(),
                            out_offset=bass.IndirectOffsetOnAxis(
                                ap=pos_int[:, k:k + 1], axis=0),
                            in_=pk[:], in_offset=None,
                            bounds_check=CAP - 1, oob_is_err=False)

            # ---- phase D: expert MLP over staged tokens ----
            with (
                tc.tile_pool(name="pd_st", bufs=8) as pst,
                tc.tile_pool(name="pd_xt", bufs=2) as pxt,
                tc.tile_pool(name="pd_w2", bufs=4) as pw2,
                tc.tile_pool(name="pd_y", bufs=1) as pyp,
                tc.tile_pool(name="pd_tok", bufs=8) as ptk,
                tc.tile_pool(name="pd_psT", bufs=2, space="PSUM") as pdT,
                tc.tile_pool(name="pd_psH", bufs=2, space="PSUM") as pdH,
                tc.tile_pool(name="pd_psY", bufs=1, space="PSUM") as pdY,
            ):
                for j in range(n_exp_tiles):
                    sts, idints, cscales = [], [], []
                    for g in range(TT // 128):
                        kk = j * (TT // 128) + g
                        if SPARSE:
                            st = pst.tile([128, ROWW], dt.float32, tag="st")
                            nc.sync.dma_start(
                                st[:], staging.ap()[kk * 128:(kk + 1) * 128, :])
                            idint = pst.tile([128, 1], dt.int32, tag="idint")
                            nc.vector.tensor_copy(idint[:], st[:, 0:1])
                            xv = st[:, 2:ROWW].bitcast(dt.bfloat16)
                            cap_ = st[:, 1:2]
                        else:
                            st = pst.tile([128, D], dt.bfloat16, tag="st")
                            nc.sync.dma_start(
                                st[:], xbf_dram.ap()[kk * 128:(kk + 1) * 128, :])
                            idint = None
                            xv = st[:]
                            cap_ = c_all[:, kk:kk + 1]
                        sts.append(st)
                        idints.append(idint)
                        cscales.append(cap_)

                        # transpose x into XT (below) per group
                    xtb = pxt.tile([128, (D // 128) * TT], dt.bfloat16, tag="xtb")
                    xtb_v = xtb[:].rearrange("p (c t) -> p c t", c=D // 128)
                    for g in range(TT // 128):
                        xv = (sts[g][:, 2:ROWW].bitcast(dt.bfloat16)
                              if SPARSE else sts[g][:])
                        for h in range(2):
                            ptx = pdT.tile([128, 512], dt.bfloat16, tag="ptt")
                            for c4 in range(4):
                                c = 4 * h + c4
                                nc.tensor.transpose(
                                    ptx[:, c4 * 128:(c4 + 1) * 128],
                                    xv[:, c * 128:(c + 1) * 128], idb[:])
                            nc.vector.tensor_copy(
                                xtb_v[:, 4 * h:4 * h + 4,
                                      g * 128:(g + 1) * 128],
                                ptx[:].rearrange("p (c t) -> p c t", c=4))

                    # h = gelu(x @ w1 + b1)   -> hT [F-chunk part, TT]
                    for f in range(F // 128):
                        ph = pdH.tile([128, TT], dt.float32, tag="ph")
                        for c in range(D // 128):
                            nc.tensor.matmul(
                                ph[:],
                                lhsT=w1bf[:, c * F + f * 128: c * F + (f + 1) * 128],
                                rhs=xtb[:, c * TT:(c + 1) * TT],
                                start=(c == 0), stop=(c == D // 128 - 1))
                        nc.scalar.activation(
                            hT[:, f * TT:(f + 1) * TT], ph[:], Act.Gelu,
                            bias=b1p[:, f:f + 1])

                    # y = hT.T @ w2 + b2, scaled by combine weight, scattered
                    ytoks = [ptk.tile([128, D], dt.bfloat16, tag="ytok",
                                      name=f"ytok_{j}_{g}")
                             for g in range(TT // 128)]
                    for hh in range(2):
                        pys = [pyp.tile([128, TT], dt.float32, tag=f"py{dd}",
                                        name=f"py_{j}_{hh}_{dd}")
                               for dd in range(4)]
                        pys_ps = [pdY.tile([128, TT], dt.float32, tag=f"psy{dd}",
                                           name=f"psy_{j}_{hh}_{dd}")
                                  for dd in range(4)]
                        for f in range(F // 128):
                            w2s = pw2.tile([128, 512], dt.bfloat16, tag="w2s")
                            nc.sync.dma_start(
                                w2s[:],
                                w2bf_dram.ap()[f * 128:(f + 1) * 128,
                                               hh * 512:(hh + 1) * 512])
                            for dd in range(4):
                                nc.tensor.matmul(
                                    pys_ps[dd][:],
                                    lhsT=w2s[:, dd * 128:(dd + 1) * 128],
                                    rhs=hT[:, f * TT:(f + 1) * TT],
                                    start=(f == 0), stop=(f == F // 128 - 1))
                        for dd in range(4):
                            d_ = 4 * hh + dd
                            nc.vector.tensor_scalar(
                                out=pys[dd][:], in0=pys_ps[dd][:],
                                scalar1=b2p[:, d_:d_ + 1], scalar2=None,
                                op0=Alu.add)
                            pty = pdT.tile([128, TT], dt.float32, tag="ptt")
                            for g in range(TT // 128):
                                nc.tensor.transpose(
                                    pty[:, g * 128:(g + 1) * 128],
                                    pys[dd][:, g * 128:(g + 1) * 128], idf[:])
                            for g in range(TT // 128):
                                nc.scalar.activation(
                                    ytoks[g][:, d_ * 128:(d_ + 1) * 128],
                                    pty[:, g * 128:(g + 1) * 128], Act.Copy,
                                    scale=cscales[g])
                    for g in range(TT // 128):
                        kk = j * (TT // 128) + g
                        if SPARSE:
                            nc.gpsimd.indirect_dma_start(
                                out=out_buf.ap(),
                                out_offset=bass.IndirectOffsetOnAxis(
                                    ap=idints[g][:, 0:1], axis=0),
                                in_=ytoks[g][:], in_offset=None,
                                bounds_check=T - 1, oob_is_err=False)
                        else:
                            nc.sync.dma_start(
                                out_buf.ap()[kk * 128:(kk + 1) * 128, :],
                                ytoks[g][:])

            # ---- phase R: combine across cores + emit shard ----
            nc.gpsimd.collective_compute(
                "ReduceScatter",
                mybir.AluOpType.add,
                replica_groups=[list(range(NCORES))],
                ins=[out_buf.ap().opt()],
                outs=[rs_out.ap().opt()],
            )
            with tc.tile_pool(name="po_sb", bufs=2) as po:
                for k in range(T // NCORES // 128):
                    ob = po.tile([128, D], dt.bfloat16, tag="ob")
                    nc.sync.dma_start(ob[:], rs_out.ap()[k * 128:(k + 1) * 128, :])
                    of = po.tile([128, D], dt.float32, tag="of")
                    nc.scalar.copy(of[:], ob[:])
                    nc.sync.dma_start(out_ext.ap()[k * 128:(k + 1) * 128, :], of[:])

    nc.compile()
    return nc


def _make_inputs(inputs):
    import ml_dtypes
    hidden = np.ascontiguousarray(
        np.asarray(inputs["hidden_states"], dtype=np.float32).reshape(T, D))
    router_w = np.asarray(inputs["router_w"], dtype=np.float32)
    w1 = np.asarray(inputs["w1"], dtype=np.float32)
    b1 = np.asarray(inputs["b1"], dtype=np.float32)
    w2 = np.asarray(inputs["w2"], dtype=np.float32)
    b2 = np.asarray(inputs["b2"], dtype=np.float32)

    # packed router weights: rw[p, 8c+j] = router_w[128c+p, j]
    rw = np.ascontiguousarray(
        router_w.reshape(8, 128, E).transpose(1, 0, 2).reshape(128, 64))
    ids = np.ascontiguousarray(
        (np.arange(T, dtype=np.float32).reshape(NT, 128).T))
    tri = np.triu(np.ones((128, 128), dtype=np.float32), k=1)
    ones = np.ones((128, 128), dtype=np.float32)
    idf = np.eye(128, dtype=np.float32)
    idb = np.eye(128, dtype=np.float32).astype(ml_dtypes.bfloat16)

    in_maps = []
    for e in range(NCORES):
        esel = np.zeros((128, E), dtype=np.float32)
        esel[:, e] = 1.0
        b1p = np.ascontiguousarray(
            b1[e].reshape(F // 128, 128).T).astype(np.float32)
        b2p = np.ascontiguousarray(
            b2[e].reshape(D // 128, 128).T).astype(np.float32)
        in_maps.append({
            "hidden": hidden,
            "rw": rw,
            "w1": np.ascontiguousarray(w1[e]),
            "b1p": b1p,
            "w2": np.ascontiguousarray(w2[e]),
            "b2p": b2p,
            "esel": esel,
            "ids": ids,
            "tri": tri,
            "ones": ones,
            "idf": idf,
            "idb": idb,
        })
    return in_maps


def kernel(**inputs):
    from concourse.bass_utils import run_bass_kernel_spmd

    if "nc" not in _STATE:
        _STATE["nc"] = _build()
    nc = _STATE["nc"]

    in_maps = _make_inputs(inputs)
    want_trace = os.environ.get("MOE_TRACE", "0") == "1"
    try:
        res = run_bass_kernel_spmd(nc, in_maps, list(range(NCORES)),
                                   trace=want_trace)
    except ModuleNotFoundError:
        res = run_bass_kernel_spmd(nc, in_maps, list(range(NCORES)),
                                   trace=False)
    _STATE["last_results"] = res

    out = np.concatenate(
        [np.asarray(res.results[c]["out"]) for c in range(NCORES)], axis=0)
    out = out.reshape(B, S, D).astype(np.float32)
    lb = np.float32(np.asarray(res.results[0]["lb"])[0, 0])
    return out, lb


# revision 17
# speedup vs baseline: 48.0779x; 48.0779x over previous
"""MoE layer (8 experts, top-2) on 8 TRN2 NeuronCores.

Strategy (expert-parallel, sparse dispatch):
  - Every core receives the full hidden_states plus ONE expert's weights
    (host-sharded across cores) and replicated router weights.
  - On device, each core computes the router (fp32 matmuls), softmax-free
    top-2 combine weights, and the load-balance loss for ALL 8192 tokens.
  - Each core compacts the tokens routed to ITS expert via a matmul-based
    prefix-sum over the selection mask, then scatters packed rows
    [token_id, combine_w, x(bf16)] into a capacity-bounded staging buffer
    with one indirect DMA per 128-token tile (OOB slots dropped).
  - The expert MLP (gelu(x@w1+b1)@w2+b2, bf16 matmuls, fp32 accumulate)
    runs over CAP staged slots; outputs are scaled by the combine weight
    and scattered back to a [8192,1024] bf16 buffer by token id.
  - A ReduceScatter(+) over the 8 cores combines the per-expert partial
    outputs; each core emits its 1024-token shard, host concatenates.
"""
import os

import numpy as np

# ---- problem constants (hardcoded per contest contract) ----
B, S, D, F, E = 4, 2048, 1024, 4096, 8
T = B * S                    # 8192 tokens
NT = T // 128                # 64 token tiles
NCORES = 8
Z_LOSS_COEF = 0.001

SPARSE = os.environ.get("MOE_SPARSE", "1") == "1"
SKIP_RS = os.environ.get("MOE_SKIP_RS", "0") == "1"       # debug: no collective
PLAIN_SCATTER = os.environ.get("MOE_PLAIN_SCATTER", "0") == "1"  # debug: no indirect DMA
MM_REDUCE = os.environ.get("MOE_MM_REDUCE", "0") == "1"   # debug: no gpsimd reduce
SKIP_EXPERT = os.environ.get("MOE_SKIP_EXPERT", "0") == "1"  # debug: no phase D
SKIP_ROUTER = os.environ.get("MOE_SKIP_ROUTER", "0") == "1"  # debug: no phase A
CAP = int(os.environ.get("MOE_CAP", "3072"))   # per-expert token capacity
TT = 512                      # expert-phase token tile
ROWW = 2 + D // 2             # packed staging row in f32 words: id, c, x(bf16)
BIGPOS = 100000.0

_STATE = {}


def _build():
    import concourse.bass as bass
    import concourse.bacc as bacc
    import concourse.mybir as mybir
    import concourse.tile as tile

    dt = mybir.dt
    Act = mybir.ActivationFunctionType
    Alu = mybir.AluOpType

    nc = bacc.Bacc("TRN2", target_bir_lowering=False, debug=False,
                   num_devices=NCORES)

    # ---- I/O ----
    hidden = nc.dram_tensor("hidden", [T, D], dt.float32, kind="ExternalInput")
    rw_in = nc.dram_tensor("rw", [128, 64], dt.float32, kind="ExternalInput")
    w1_in = nc.dram_tensor("w1", [D, F], dt.float32, kind="ExternalInput")
    b1_in = nc.dram_tensor("b1p", [128, F // 128], dt.float32, kind="ExternalInput")
    w2_in = nc.dram_tensor("w2", [F, D], dt.float32, kind="ExternalInput")
    b2_in = nc.dram_tensor("b2p", [128, D // 128], dt.float32, kind="ExternalInput")
    esel_in = nc.dram_tensor("esel", [128, E], dt.float32, kind="ExternalInput")
    ids_in = nc.dram_tensor("ids", [128, NT], dt.float32, kind="ExternalInput")
    tri_in = nc.dram_tensor("tri", [128, 128], dt.float32, kind="ExternalInput")
    ones_in = nc.dram_tensor("ones", [128, 128], dt.float32, kind="ExternalInput")
    idf_in = nc.dram_tensor("idf", [128, 128], dt.float32, kind="ExternalInput")
    idb_in = nc.dram_tensor("idb", [128, 128], dt.bfloat16, kind="ExternalInput")

    out_ext = nc.dram_tensor("out", [T // NCORES, D], dt.float32, kind="ExternalOutput")
    lb_ext = nc.dram_tensor("lb", [1, 1], dt.float32, kind="ExternalOutput")

    # ---- internal DRAM ----
    xbf_dram = nc.dram_tensor("xbf_dram", [T, D], dt.bfloat16)
    w2bf_dram = nc.dram_tensor("w2bf_dram", [F, D], dt.bfloat16)
    out_buf = nc.dram_tensor("out_buf", [T, D], dt.bfloat16)
    rs_out = nc.dram_tensor("rs_out", [T // NCORES, D], dt.bfloat16)
    if SPARSE:
        staging = nc.dram_tensor("staging", [CAP, ROWW], dt.float32)

    n_exp_tiles = (CAP if SPARSE else T) // TT

    with tile.TileContext(nc) as tc:
        with tc.tile_pool(name="const", bufs=1) as cp:
            # ---- persistent tiles ----
            rw = cp.tile([128, 64], dt.float32, tag="rw")
            esel = cp.tile([128, E], dt.float32, tag="esel")
            ids = cp.tile([128, NT], dt.float32, tag="ids")
            tri = cp.tile([128, 128], dt.float32, tag="tri")
            ones = cp.tile([128, 128], dt.float32, tag="ones")
            idf = cp.tile([128, 128], dt.float32, tag="idf")
            idb = cp.tile([128, 128], dt.bfloat16, tag="idb")
            b1p = cp.tile([128, F // 128], dt.float32, tag="b1p")
            b2p = cp.tile([128, D // 128], dt.float32, tag="b2p")
            for t_, src in ((rw, rw_in), (esel, esel_in), (ids, ids_in),
                            (tri, tri_in), (ones, ones_in), (idf, idf_in),
                            (idb, idb_in), (b1p, b1_in), (b2p, b2_in)):
                nc.sync.dma_start(t_[:], src.ap())

            c_all = cp.tile([128, NT], dt.float32, tag="c_all")
            acc = cp.tile([128, E], dt.float32, tag="acc")
            nc.vector.memset(acc[:], 0.0)

            w1bf = cp.tile([128, D // 128 * F], dt.bfloat16, tag="w1bf")
            hT = cp.tile([128, F // 128 * TT], dt.bfloat16, tag="hT")
            pos_int = cp.tile([128, NT], dt.int32, tag="pos_int")

            # ---- phase W: weight conversion ----
            with tc.tile_pool(name="wconv", bufs=2) as wp:
                for c in range(D // 128):
                    st = wp.tile([128, F], dt.float32, tag="w1st")
                    nc.sync.dma_start(st[:], w1_in.ap()[c * 128:(c + 1) * 128, :])
                    nc.scalar.copy(w1bf[:, c * F:(c + 1) * F], st[:])
                for c in range(F // 128):
                    st2 = wp.tile([128, D], dt.float32, tag="w2st")
                    nc.sync.dma_start(st2[:], w2_in.ap()[c * 128:(c + 1) * 128, :])
                    st2b = wp.tile([128, D], dt.bfloat16, tag="w2stb")
                    nc.vector.tensor_copy(st2b[:], st2[:])
                    nc.sync.dma_start(w2bf_dram.ap()[c * 128:(c + 1) * 128, :], st2b[:])

            # zero-fill the output scatter buffer (only scattered rows get data)
            zrow = cp.tile([128, D], dt.bfloat16, tag="zrow")
            nc.vector.memset(zrow[:], 0.0)
            if SPARSE:
                for k in range(NT):
                    nc.sync.dma_start(out_buf.ap()[k * 128:(k + 1) * 128, :], zrow[:])

            # ---- phase A: router over all tokens ----
            if SKIP_ROUTER:
                nc.vector.memset(c_all[:], 0.3)
            with (
                tc.tile_pool(name="pa_sb", bufs=3) as pa,
                tc.tile_pool(name="pa_small", bufs=4) as pas,
                tc.tile_pool(name="pa_psT", bufs=2, space="PSUM") as paT,
                tc.tile_pool(name="pa_psL", bufs=2, space="PSUM") as paL,
            ):
                for k in range(0 if SKIP_ROUTER else NT):
                    x = pa.tile([128, D], dt.float32, tag="x")
                    nc.sync.dma_start(x[:], hidden.ap()[k * 128:(k + 1) * 128, :])
                    xb = pa.tile([128, D], dt.bfloat16, tag="xb")
                    nc.scalar.copy(xb[:], x[:])
                    nc.sync.dma_start(xbf_dram.ap()[k * 128:(k + 1) * 128, :], xb[:])

                    xt = pa.tile([128, D], dt.float32, tag="xt")
                    for h in range(2):
                        pt = paT.tile([128, 512], dt.float32, tag="pt")
                        for c4 in range(4):
                            c = 4 * h + c4
                            nc.tensor.transpose(
                                pt[:, c4 * 128:(c4 + 1) * 128],
                                x[:, c * 128:(c + 1) * 128], idf[:])
                        nc.vector.tensor_copy(xt[:, h * 512:(h + 1) * 512], pt[:])

                    pl = paL.tile([128, E], dt.float32, tag="pl")
                    for c in range(8):
                        nc.tensor.matmul(
                            pl[:], lhsT=xt[:, c * 128:(c + 1) * 128],
                            rhs=rw[:, c * 8:(c + 1) * 8],
                            start=(c == 0), stop=(c == 7))

                    ex = pas.tile([128, E], dt.float32, tag="ex")
                    ssum = pas.tile([128, 1], dt.float32, tag="ssum")
                    nc.scalar.activation(ex[:], pl[:], Act.Exp, accum_out=ssum[:])
                    srec = pas.tile([128, 1], dt.float32, tag="srec")
                    nc.vector.reciprocal(srec[:], ssum[:])
                    m8 = pas.tile([128, 8], dt.float32, tag="m8")
                    nc.vector.max(m8[:], ex[:])
                    dn = pas.tile([128, 1], dt.float32, tag="dn")
                    nc.vector.tensor_tensor(
                        out=dn[:], in0=m8[:, 0:1], in1=m8[:, 1:2], op=Alu.add)
                    rd = pas.tile([128, 1], dt.float32, tag="rd")
                    nc.vector.reciprocal(rd[:], dn[:])
                    cu = pas.tile([128, E], dt.float32, tag="cu")
                    nc.vector.scalar_tensor_tensor(
                        out=cu[:], in0=ex[:], scalar=m8[:, 1:2], in1=ex[:],
                        op0=Alu.is_ge, op1=Alu.mult)
                    scr = pas.tile([128, E], dt.float32, tag="scr")
                    nc.vector.scalar_tensor_tensor(
                        out=scr[:], in0=cu[:], scalar=rd[:], in1=esel[:],
                        op0=Alu.mult, op1=Alu.mult,
                        accum_out=c_all[:, k:k + 1])
                    nc.vector.scalar_tensor_tensor(
                        out=acc[:], in0=ex[:], scalar=srec[:], in1=acc[:],
                        op0=Alu.mult, op1=Alu.add)

            # ---- load-balance loss ----
            with (
                tc.tile_pool(name="lb_sb", bufs=1) as lp,
                tc.tile_pool(name="lb_ps", bufs=1, space="PSUM") as lpp,
            ):
                loadr = lp.tile([1, E], dt.float32, tag="loadr")
                if MM_REDUCE:
                    lps = lpp.tile([1, E], dt.float32, tag="lps")
                    nc.tensor.matmul(lps[:], lhsT=ones[:, 0:1], rhs=acc[:],
                                     start=True, stop=True)
                    nc.vector.tensor_copy(loadr[:], lps[:])
                else:
                    nc.gpsimd.tensor_reduce(
                        out=loadr[:], in_=acc[:], axis=mybir.AxisListType.C,
                        op=Alu.add)
                nc.vector.tensor_scalar(
                    out=loadr[:], in0=loadr[:], scalar1=1.0 / T,
                    scalar2=-1.0 / E, op0=Alu.mult, op1=Alu.add)
                sq = lp.tile([1, E], dt.float32, tag="sq")
                nc.scalar.square(sq[:], loadr[:])
                lbv = lp.tile([1, 1], dt.float32, tag="lbv")
                nc.vector.tensor_reduce(
                    out=lbv[:], in_=sq[:], axis=mybir.AxisListType.X, op=Alu.add)
                nc.vector.tensor_scalar(
                    out=lbv[:], in0=lbv[:], scalar1=Z_LOSS_COEF / E,
                    scalar2=None, op0=Alu.mult)
                nc.sync.dma_start(lb_ext.ap(), lbv[:])

            if SPARSE:
                # ---- phase B: compaction (positions via matmul prefix-sum) ----
                with (
                    tc.tile_pool(name="pb_sb", bufs=1) as pb,
                    tc.tile_pool(name="pb_ps", bufs=1, space="PSUM") as pbp,
                ):
                    mm_ = pb.tile([128, NT], dt.float32, tag="mm")
                    nc.vector.tensor_scalar(
                        out=mm_[:], in0=c_all[:], scalar1=0.0, scalar2=None,
                        op0=Alu.is_gt)
                    cs_ps = pbp.tile([NT, 1], dt.float32, tag="cs_ps")
                    nc.tensor.matmul(cs_ps[:], lhsT=mm_[:], rhs=ones[:, 0:1],
                                     start=True, stop=True)
                    cs = pb.tile([NT, 1], dt.float32, tag="cs")
                    nc.vector.tensor_copy(cs[:], cs_ps[:])
                    tric = pb.tile([NT, NT], dt.float32, tag="tric")
                    nc.vector.tensor_scalar(
                        out=tric[:], in0=tri[:NT, :NT], scalar1=cs[:, 0:1],
                        scalar2=None, op0=Alu.mult)
                    pos_ps = pbp.tile([128, NT], dt.float32, tag="pos_ps")
                    nc.tensor.matmul(pos_ps[:], lhsT=ones[:NT, :],
                                     rhs=tric[:], start=True, stop=False)
                    nc.tensor.matmul(pos_ps[:], lhsT=tri[:], rhs=mm_[:],
                                     start=False, stop=True)
                    posf = pb.tile([128, NT], dt.float32, tag="posf")
                    # pos + (1-mask)*BIG  ==  (mask*-BIG + pos) + BIG
                    nc.vector.scalar_tensor_tensor(
                        out=posf[:], in0=mm_[:], scalar=-BIGPOS, in1=pos_ps[:],
                        op0=Alu.mult, op1=Alu.add)
                    nc.vector.tensor_scalar(
                        out=posf[:], in0=posf[:], scalar1=BIGPOS, scalar2=None,
                        op0=Alu.add)
                    nc.vector.tensor_copy(pos_int[:], posf[:])

                # ---- phase C: pack + scatter tokens to staging ----
                with tc.tile_pool(name="pc_sb", bufs=3) as pc:
                    padrow = pc.tile([128, ROWW], dt.float32, tag="padrow")
                    nc.vector.memset(padrow[:], 0.0)
                    nc.vector.memset(padrow[:, 0:1], float(T))
                    for g in range(CAP // 128):
                        nc.sync.dma_start(
                            staging.ap()[g * 128:(g + 1) * 128, :], padrow[:])
                    for k in range(NT):
                        pk = pc.tile([128, ROWW], dt.float32, tag="pk")
                        nc.sync.dma_start(
                            pk[:, 2:ROWW].bitcast(dt.bfloat16),
                            xbf_dram.ap()[k * 128:(k + 1) * 128, :])
                        nc.vector.tensor_copy(pk[:, 0:1], ids[:, k:k + 1])
                        nc.vector.tensor_copy(pk[:, 1:2], c_all[:, k:k + 1])
                        if PLAIN_SCATTER:
                            kk = k % (CAP // 128)
                            nc.sync.dma_start(
                                staging.ap()[kk * 128:(kk + 1) * 128, :], pk[:])
                        else:
                            nc.gpsimd.indirect_dma_start(
                                out=staging.ap(),
                                out_offset=bass.IndirectOffsetOnAxis(
                                    ap=pos_int[:, k:k + 1], axis=0),
                                in_=pk[:], in_offset=None,
                                bounds_check=CAP - 1, oob_is_err=False)

            # ---- phase D: expert MLP over staged tokens ----
            with (
                tc.tile_pool(name="pd_st", bufs=8) as pst,
                tc.tile_pool(name="pd_xt", bufs=2) as pxt,
                tc.tile_pool(name="pd_w2", bufs=4) as pw2,
                tc.tile_pool(name="pd_y", bufs=1) as pyp,
                tc.tile_pool(name="pd_tok", bufs=8) as ptk,
                tc.tile_pool(name="pd_psT", bufs=2, space="PSUM") as pdT,
                tc.tile_pool(name="pd_psH", bufs=2, space="PSUM") as pdH,
                tc.tile_pool(name="pd_psY", bufs=1, space="PSUM") as pdY,
            ):
                for j in range(0 if SKIP_EXPERT else n_exp_tiles):
                    sts, idints, cscales = [], [], []
                    for g in range(TT // 128):
                        kk = j * (TT // 128) + g
                        if SPARSE:
                            st = pst.tile([128, ROWW], dt.float32, tag="st")
                            nc.sync.dma_start(
                                st[:], staging.ap()[kk * 128:(kk + 1) * 128, :])
                            idint = pst.tile([128, 1], dt.int32, tag="idint")
                            nc.vector.tensor_copy(idint[:], st[:, 0:1])
                            xv = st[:, 2:ROWW].bitcast(dt.bfloat16)
                            cap_ = st[:, 1:2]
                        else:
                            st = pst.tile([128, D], dt.bfloat16, tag="st")
                            nc.sync.dma_start(
                                st[:], xbf_dram.ap()[kk * 128:(kk + 1) * 128, :])
                            idint = None
                            xv = st[:]
                            cap_ = c_all[:, kk:kk + 1]
                        sts.append(st)
                        idints.append(idint)
                        cscales.append(cap_)

                        # transpose x into XT (below) per group
                    xtb = pxt.tile([128, (D // 128) * TT], dt.bfloat16, tag="xtb")
                    xtb_v = xtb[:].rearrange("p (c t) -> p c t", c=D // 128)
                    for g in range(TT // 128):
                        xv = (sts[g][:, 2:ROWW].bitcast(dt.bfloat16)
                              if SPARSE else sts[g][:])
                        for h in range(2):
                            ptx = pdT.tile([128, 512], dt.bfloat16, tag="ptt")
                            for c4 in range(4):
                                c = 4 * h + c4
                                nc.tensor.transpose(
                                    ptx[:, c4 * 128:(c4 + 1) * 128],
                                    xv[:, c * 128:(c + 1) * 128], idb[:])
                            nc.vector.tensor_copy(
                                xtb_v[:, 4 * h:4 * h + 4,
                                      g * 128:(g + 1) * 128],
                                ptx[:].rearrange("p (c t) -> p c t", c=4))

                    # h = gelu(x @ w1 + b1)   -> hT [F-chunk part, TT]
                    for f in range(F // 128):
                        ph = pdH.tile([128, TT], dt.float32, tag="ph")
                        for c in range(D // 128):
                            nc.tensor.matmul(
                                ph[:],
                                lhsT=w1bf[:, c * F + f * 128: c * F + (f + 1) * 128],
                                rhs=xtb[:, c * TT:(c + 1) * TT],
                                start=(c == 0), stop=(c == D // 128 - 1))
                        nc.scalar.activation(
                            hT[:, f * TT:(f + 1) * TT], ph[:], Act.Gelu,
                            bias=b1p[:, f:f + 1])

                    # y = hT.T @ w2 + b2, scaled by combine weight, scattered
                    ytoks = [ptk.tile([128, D], dt.bfloat16, tag="ytok",
                                      name=f"ytok_{j}_{g}")
                             for g in range(TT // 128)]
                    for hh in range(2):
                        pys = [pyp.tile([128, TT], dt.float32, tag=f"py{dd}",
                                        name=f"py_{j}_{hh}_{dd}")
                               for dd in range(4)]
                        pys_ps = [pdY.tile([128, TT], dt.float32, tag=f"psy{dd}",
                                           name=f"psy_{j}_{hh}_{dd}")
                                  for dd in range(4)]
                        for f in range(F // 128):
                            w2s = pw2.tile([128, 512], dt.bfloat16, tag="w2s")
                            nc.sync.dma_start(
                                w2s[:],
                                w2bf_dram.ap()[f * 128:(f + 1) * 128,
                                               hh * 512:(hh + 1) * 512])
                            for dd in range(4):
                                nc.tensor.matmul(
                                    pys_ps[dd][:],
                                    lhsT=w2s[:, dd * 128:(dd + 1) * 128],
                                    rhs=hT[:, f * TT:(f + 1) * TT],
                                    start=(f == 0), stop=(f == F // 128 - 1))
                        for dd in range(4):
                            d_ = 4 * hh + dd
                            nc.vector.tensor_scalar(
                                out=pys[dd][:], in0=pys_ps[dd][:],
                                scalar1=b2p[:, d_:d_ + 1], scalar2=None,
                                op0=Alu.add)
                            pty = pdT.tile([128, TT], dt.float32, tag="ptt")
                            for g in range(TT // 128):
                                nc.tensor.transpose(
                                    pty[:, g * 128:(g + 1) * 128],
                                    pys[dd][:, g * 128:(g + 1) * 128], idf[:])
                            for g in range(TT // 128):
                                nc.scalar.activation(
                                    ytoks[g][:, d_ * 128:(d_ + 1) * 128],
                                    pty[:, g * 128:(g + 1) * 128], Act.Copy,
                                    scale=cscales[g])
                    for g in range(TT // 128):
                        kk = j * (TT // 128) + g
                        if SPARSE and not PLAIN_SCATTER:
                            nc.gpsimd.indirect_dma_start(
                                out=out_buf.ap(),
                                out_offset=bass.IndirectOffsetOnAxis(
                                    ap=idints[g][:, 0:1], axis=0),
                                in_=ytoks[g][:], in_offset=None,
                                bounds_check=T - 1, oob_is_err=False)
                        else:
                            nc.sync.dma_start(
                                out_buf.ap()[kk * 128:(kk + 1) * 128, :],
                                ytoks[g][:])

            # ---- phase R: combine across cores + emit shard ----
            if SKIP_RS:
                nc.sync.dma_start(rs_out.ap(), out_buf.ap()[:T // NCORES, :])
            else:
                nc.gpsimd.collective_compute(
                    "ReduceScatter",
                    mybir.AluOpType.add,
                    replica_groups=[list(range(NCORES))],
                    ins=[out_buf.ap().opt()],
                    outs=[rs_out.ap().opt()],
                )
            with tc.tile_pool(name="po_sb", bufs=2) as po:
                for k in range(T // NCORES // 128):
                    ob = po.tile([128, D], dt.bfloat16, tag="ob")
                    nc.sync.dma_start(ob[:], rs_out.ap()[k * 128:(k + 1) * 128, :])
                    of = po.tile([128, D], dt.float32, tag="of")
                    nc.scalar.copy(of[:], ob[:])
                    nc.sync.dma_start(out_ext.ap()[k * 128:(k + 1) * 128, :], of[:])

    nc.compile()
    return nc


def _make_inputs(inputs):
    import ml_dtypes
    hidden = np.ascontiguousarray(
        np.asarray(inputs["hidden_states"], dtype=np.float32).reshape(T, D))
    router_w = np.asarray(inputs["router_w"], dtype=np.float32)
    w1 = np.asarray(inputs["w1"], dtype=np.float32)
    b1 = np.asarray(inputs["b1"], dtype=np.float32)
    w2 = np.asarray(inputs["w2"], dtype=np.float32)
    b2 = np.asarray(inputs["b2"], dtype=np.float32)

    # packed router weights: rw[p, 8c+j] = router_w[128c+p, j]
    rw = np.ascontiguousarray(
        router_w.reshape(8, 128, E).transpose(1, 0, 2).reshape(128, 64))
    ids = np.ascontiguousarray(
        (np.arange(T, dtype=np.float32).reshape(NT, 128).T))
    tri = np.triu(np.ones((128, 128), dtype=np.float32), k=1)
    ones = np.ones((128, 128), dtype=np.float32)
    idf = np.eye(128, dtype=np.float32)
    idb = np.eye(128, dtype=np.float32).astype(ml_dtypes.bfloat16)

    in_maps = []
    for e in range(NCORES):
        esel = np.zeros((128, E), dtype=np.float32)
        esel[:, e] = 1.0
        b1p = np.ascontiguousarray(
            b1[e].reshape(F // 128, 128).T).astype(np.float32)
        b2p = np.ascontiguousarray(
            b2[e].reshape(D // 128, 128).T).astype(np.float32)
        in_maps.append({
            "hidden": hidden,
            "rw": rw,
            "w1": np.ascontiguousarray(w1[e]),
            "b1p": b1p,
            "w2": np.ascontiguousarray(w2[e]),
            "b2p": b2p,
            "esel": esel,
            "ids": ids,
            "tri": tri,
            "ones": ones,
            "idf": idf,
            "idb": idb,
        })
    return in_maps


def kernel(**inputs):
    from concourse.bass_utils import run_bass_kernel_spmd

    if "nc" not in _STATE:
        _STATE["nc"] = _build()
    nc = _STATE["nc"]

    in_maps = _make_inputs(inputs)
    want_trace = os.environ.get("MOE_TRACE", "0") == "1"
    try:
        res = run_bass_kernel_spmd(nc, in_maps, list(range(NCORES)),
                                   trace=want_trace)
    except ModuleNotFoundError:
        res = run_bass_kernel_spmd(nc, in_maps, list(range(NCORES)),
                                   trace=False)
    _STATE["last_results"] = res

    out = np.concatenate(
        [np.asarray(res.results[c]["out"]) for c in range(NCORES)], axis=0)
    out = out.reshape(B, S, D).astype(np.float32)
    lb = np.float32(np.asarray(res.results[0]["lb"])[0, 0])
    return out, lb


# revision 21
# speedup vs baseline: 58.5590x; 1.2180x over previous
"""MoE layer (8 experts, top-2) on 8 TRN2 NeuronCores.

Expert-parallel with sparse token dispatch:
  - Every core receives the full hidden_states plus ONE expert's weights
    (host-sharded across cores) and replicated router weights.
  - Phase A (per core): stream token tiles; fp32 router matmuls (via PE
    transposes); exp/top-2 combine weights; a running matmul prefix-sum
    turns this expert's selection mask into compacted slot positions; a
    packed row [token_id, combine_w, pad, x(bf16)] is scattered into a
    capacity-bounded staging buffer by indirect DMA (OOB slots dropped).
  - Phase D: the expert MLP gelu(x@w1+b1)@w2+b2 runs over CAP staged
    slots in bf16 (fp32 accumulate); X and Y are transposed with the DMA
    x-bar; outputs are scaled by the combine weight and scattered back to
    a zeroed [8192,1024] bf16 buffer by token id.
  - ReduceScatter(+) over the 8 cores combines per-expert partials; each
    core emits its 1024-token shard; host concatenates.
  - Load-balance loss from softmax-prob partial sums (identical on every
    core).
"""
import os

import numpy as np

# ---- problem constants (hardcoded per contest contract) ----
B, S, D, F, E = 4, 2048, 1024, 4096, 8
T = B * S                    # 8192 tokens
NT = T // 128                # 64 token tiles
NCORES = 8
Z_LOSS_COEF = 0.001

SPARSE = os.environ.get("MOE_SPARSE", "1") == "1"
SKIP_RS = os.environ.get("MOE_SKIP_RS", "0") == "1"       # debug: no collective
SKIP_EXPERT = os.environ.get("MOE_SKIP_EXPERT", "0") == "1"  # debug
SKIP_ROUTER = os.environ.get("MOE_SKIP_ROUTER", "0") == "1"  # debug
CAP = int(os.environ.get("MOE_CAP", "2560"))   # per-expert token capacity
TT = 512                      # expert-phase token tile
XOFF = 16                     # f32 words before x payload (32B-align for xbar)
ROWW = XOFF + D // 2          # packed staging row in f32 words
BIGPOS = 100000.0

_STATE = {}


def _build():
    import concourse.bass as bass
    import concourse.bacc as bacc
    import concourse.mybir as mybir
    import concourse.tile as tile

    dt = mybir.dt
    Act = mybir.ActivationFunctionType
    Alu = mybir.AluOpType

    nc = bacc.Bacc("TRN2", target_bir_lowering=False, debug=False,
                   num_devices=NCORES)

    # ---- I/O ----
    hidden = nc.dram_tensor("hidden", [T, D], dt.float32, kind="ExternalInput")
    rw_in = nc.dram_tensor("rw", [128, 64], dt.float32, kind="ExternalInput")
    w1_in = nc.dram_tensor("w1", [D, F], dt.float32, kind="ExternalInput")
    b1_in = nc.dram_tensor("b1p", [128, F // 128], dt.float32, kind="ExternalInput")
    w2_in = nc.dram_tensor("w2", [F, D], dt.float32, kind="ExternalInput")
    b2_in = nc.dram_tensor("b2p", [128, D // 128], dt.float32, kind="ExternalInput")
    esel_in = nc.dram_tensor("esel", [128, E], dt.float32, kind="ExternalInput")
    ids_in = nc.dram_tensor("ids", [128, NT], dt.float32, kind="ExternalInput")
    tri_in = nc.dram_tensor("tri", [128, 128], dt.float32, kind="ExternalInput")
    ones_in = nc.dram_tensor("ones", [128, 128], dt.float32, kind="ExternalInput")
    idf_in = nc.dram_tensor("idf", [128, 128], dt.float32, kind="ExternalInput")

    out_ext = nc.dram_tensor("out", [T // NCORES, D], dt.float32, kind="ExternalOutput")
    lb_ext = nc.dram_tensor("lb", [1, 1], dt.float32, kind="ExternalOutput")

    # ---- internal DRAM ----
    w2bf_dram = nc.dram_tensor("w2bf_dram", [F, D], dt.bfloat16)
    out_buf = nc.dram_tensor("out_buf", [T, D], dt.bfloat16)
    rs_out = nc.dram_tensor("rs_out", [T // NCORES, D], dt.bfloat16)
    if SPARSE:
        staging = nc.dram_tensor("staging", [CAP, ROWW], dt.float32)
    else:
        staging = nc.dram_tensor("staging", [T, ROWW], dt.float32)

    n_exp_tiles = (CAP if SPARSE else T) // TT

    with tile.TileContext(nc) as tc:
        with tc.tile_pool(name="const", bufs=1) as cp:
            # ---- persistent tiles ----
            rw = cp.tile([128, 64], dt.float32, tag="rw")
            esel = cp.tile([128, E], dt.float32, tag="esel")
            ids = cp.tile([128, NT], dt.float32, tag="ids")
            tri = cp.tile([128, 128], dt.float32, tag="tri")
            ones = cp.tile([128, 128], dt.float32, tag="ones")
            idf = cp.tile([128, 128], dt.float32, tag="idf")
            b1p = cp.tile([128, F // 128], dt.float32, tag="b1p")
            b2p = cp.tile([128, D // 128], dt.float32, tag="b2p")
            for t_, src in ((rw, rw_in), (esel, esel_in), (ids, ids_in),
                            (tri, tri_in), (ones, ones_in), (idf, idf_in),
                            (b1p, b1_in), (b2p, b2_in)):
                nc.sync.dma_start(t_[:], src.ap())

            acc = cp.tile([128, E], dt.float32, tag="acc")
            nc.vector.memset(acc[:], 0.0)
            base = cp.tile([1, 1], dt.float32, tag="base")
            nc.vector.memset(base[:], 0.0)

            w1bf = cp.tile([128, D // 128 * F], dt.bfloat16, tag="w1bf")
            hT = cp.tile([128, F // 128 * TT], dt.bfloat16, tag="hT")

            # ---- phase W: weight conversion ----
            with tc.tile_pool(name="wconv", bufs=2) as wp:
                for c in range(D // 128):
                    st = wp.tile([128, F], dt.float32, tag="w1st")
                    nc.sync.dma_start(st[:], w1_in.ap()[c * 128:(c + 1) * 128, :])
                    nc.scalar.copy(w1bf[:, c * F:(c + 1) * F], st[:])
                for c in range(F // 128):
                    st2 = wp.tile([128, D], dt.float32, tag="w2st")
                    nc.sync.dma_start(st2[:], w2_in.ap()[c * 128:(c + 1) * 128, :])
                    st2b = wp.tile([128, D], dt.bfloat16, tag="w2stb")
                    nc.vector.tensor_copy(st2b[:], st2[:])
                    nc.sync.dma_start(w2bf_dram.ap()[c * 128:(c + 1) * 128, :], st2b[:])

            # zero-fill output scatter buffer + pad-fill staging
            zrow = cp.tile([128, D], dt.bfloat16, tag="zrow")
            nc.vector.memset(zrow[:], 0.0)
            if SPARSE:
                for k in range(NT):
                    nc.sync.dma_start(out_buf.ap()[k * 128:(k + 1) * 128, :], zrow[:])
                padrow = cp.tile([128, ROWW], dt.float32, tag="padrow")
                nc.vector.memset(padrow[:], 0.0)
                nc.vector.memset(padrow[:, 0:1], float(T))
                for g in range(CAP // 128):
                    nc.sync.dma_start(
                        staging.ap()[g * 128:(g + 1) * 128, :], padrow[:])

            # ---- phase A: router + dispatch over all tokens ----
            with (
                tc.tile_pool(name="pa_sb", bufs=3) as pa,
                tc.tile_pool(name="pa_pk", bufs=3) as ppk,
                tc.tile_pool(name="pa_small", bufs=4) as pas,
                tc.tile_pool(name="pa_psT", bufs=2, space="PSUM") as paT,
                tc.tile_pool(name="pa_psL", bufs=2, space="PSUM") as paL,
                tc.tile_pool(name="pa_psP", bufs=1, space="PSUM") as paP,
            ):
                for k in range(0 if SKIP_ROUTER else NT):
                    x = pa.tile([128, D], dt.float32, tag="x")
                    nc.sync.dma_start(x[:], hidden.ap()[k * 128:(k + 1) * 128, :])

                    xt = pa.tile([128, D], dt.float32, tag="xt")
                    for h in range(2):
                        pt = paT.tile([128, 512], dt.float32, tag="pt")
                        for c4 in range(4):
                            c = 4 * h + c4
                            nc.tensor.transpose(
                                pt[:, c4 * 128:(c4 + 1) * 128],
                                x[:, c * 128:(c + 1) * 128], idf[:])
                        nc.vector.tensor_copy(xt[:, h * 512:(h + 1) * 512], pt[:])

                    pl = paL.tile([128, E], dt.float32, tag="pl")
                    for c in range(8):
                        nc.tensor.matmul(
                            pl[:], lhsT=xt[:, c * 128:(c + 1) * 128],
                            rhs=rw[:, c * 8:(c + 1) * 8],
                            start=(c == 0), stop=(c == 7))

                    ex = pas.tile([128, E], dt.float32, tag="ex")
                    ssum = pas.tile([128, 1], dt.float32, tag="ssum")
                    nc.scalar.activation(ex[:], pl[:], Act.Exp, accum_out=ssum[:])
                    srec = pas.tile([128, 1], dt.float32, tag="srec")
                    nc.vector.reciprocal(srec[:], ssum[:])
                    m8 = pas.tile([128, 8], dt.float32, tag="m8")
                    nc.vector.max(m8[:], ex[:])
                    dn = pas.tile([128, 1], dt.float32, tag="dn")
                    nc.vector.tensor_tensor(
                        out=dn[:], in0=m8[:, 0:1], in1=m8[:, 1:2], op=Alu.add)
                    rd = pas.tile([128, 1], dt.float32, tag="rd")
                    nc.vector.reciprocal(rd[:], dn[:])
                    cu = pas.tile([128, E], dt.float32, tag="cu")
                    nc.vector.scalar_tensor_tensor(
                        out=cu[:], in0=ex[:], scalar=m8[:, 1:2], in1=ex[:],
                        op0=Alu.is_ge, op1=Alu.mult)
                    # packed row: [id, c, pad..., x bf16]
                    pk = ppk.tile([128, ROWW], dt.float32, tag="pk")
                    nc.vector.memset(pk[:, 2:XOFF], 0.0)
                    scr = pas.tile([128, E], dt.float32, tag="scr")
                    nc.vector.scalar_tensor_tensor(
                        out=scr[:], in0=cu[:], scalar=rd[:], in1=esel[:],
                        op0=Alu.mult, op1=Alu.mult,
                        accum_out=pk[:, 1:2])
                    nc.vector.scalar_tensor_tensor(
                        out=acc[:], in0=ex[:], scalar=srec[:], in1=acc[:],
                        op0=Alu.mult, op1=Alu.add)

                    if SPARSE:
                        mask = pas.tile([128, 1], dt.float32, tag="mask")
                        nc.vector.tensor_scalar(
                            out=mask[:], in0=pk[:, 1:2], scalar1=0.0,
                            scalar2=None, op0=Alu.is_gt)
                        # slot = (# selected before this token) + running base
                        pos_ps = paP.tile([128, 1], dt.float32, tag="pos_ps")
                        nc.tensor.matmul(pos_ps[:], lhsT=tri[:], rhs=mask[:],
                                         start=True, stop=False)
                        nc.tensor.matmul(pos_ps[:], lhsT=ones[0:1, :],
                                         rhs=base[:], start=False, stop=True)
                        cs_ps = paP.tile([1, 1], dt.float32, tag="cs_ps")
                        nc.tensor.matmul(cs_ps[:], lhsT=mask[:],
                                         rhs=ones[:, 0:1], start=True, stop=True)
                        nc.vector.tensor_tensor(
                            out=base[:], in0=base[:], in1=cs_ps[:], op=Alu.add)
                        posf = pas.tile([128, 1], dt.float32, tag="posf")
                        nc.vector.scalar_tensor_tensor(
                            out=posf[:], in0=mask[:], scalar=-BIGPOS,
                            in1=pos_ps[:], op0=Alu.mult, op1=Alu.add)
                        nc.vector.tensor_scalar(
                            out=posf[:], in0=posf[:], scalar1=BIGPOS,
                            scalar2=None, op0=Alu.add)
                        posi = pas.tile([128, 1], dt.int32, tag="posi")
                        nc.vector.tensor_copy(posi[:], posf[:])

                        nc.vector.tensor_copy(pk[:, 0:1], ids[:, k:k + 1])
                        nc.vector.tensor_copy(
                            pk[:, XOFF:ROWW].bitcast(dt.bfloat16), x[:])
                        nc.gpsimd.indirect_dma_start(
                            out=staging.ap(),
                            out_offset=bass.IndirectOffsetOnAxis(
                                ap=posi[:, 0:1], axis=0),
                            in_=pk[:], in_offset=None,
                            bounds_check=CAP - 1, oob_is_err=False)
                    else:
                        nc.vector.tensor_copy(pk[:, 0:1], ids[:, k:k + 1])
                        nc.vector.tensor_copy(
                            pk[:, XOFF:ROWW].bitcast(dt.bfloat16), x[:])
                        nc.sync.dma_start(
                            staging.ap()[k * 128:(k + 1) * 128, :], pk[:])

            # ---- load-balance loss ----
            with (
                tc.tile_pool(name="lb_sb", bufs=1) as lp,
                tc.tile_pool(name="lb_ps", bufs=1, space="PSUM") as lpp,
            ):
                loadr = lp.tile([1, E], dt.float32, tag="loadr")
                lps = lpp.tile([1, E], dt.float32, tag="lps")
                nc.tensor.matmul(lps[:], lhsT=ones[:, 0:1], rhs=acc[:],
                                 start=True, stop=True)
                nc.vector.tensor_scalar(
                    out=loadr[:], in0=lps[:], scalar1=1.0 / T,
                    scalar2=-1.0 / E, op0=Alu.mult, op1=Alu.add)
                sq = lp.tile([1, E], dt.float32, tag="sq")
                nc.vector.tensor_tensor(out=sq[:], in0=loadr[:], in1=loadr[:],
                                        op=Alu.mult)
                lbv = lp.tile([1, 1], dt.float32, tag="lbv")
                nc.vector.tensor_reduce(
                    out=lbv[:], in_=sq[:], axis=mybir.AxisListType.X, op=Alu.add)
                nc.vector.tensor_scalar(
                    out=lbv[:], in0=lbv[:], scalar1=Z_LOSS_COEF / E,
                    scalar2=None, op0=Alu.mult)
                nc.sync.dma_start(lb_ext.ap(), lbv[:])

            # ---- phase D: expert MLP over staged tokens ----
            with (
                tc.tile_pool(name="pd_st", bufs=8) as pst,
                tc.tile_pool(name="pd_xt", bufs=2) as pxt,
                tc.tile_pool(name="pd_w2", bufs=4) as pw2,
                tc.tile_pool(name="pd_yb", bufs=4) as pyb,
                tc.tile_pool(name="pd_tok", bufs=2) as ptk,
                tc.tile_pool(name="pd_psH", bufs=2, space="PSUM") as pdH,
                tc.tile_pool(name="pd_psY", bufs=1, space="PSUM") as pdY,
            ):
                for j in range(0 if SKIP_EXPERT else n_exp_tiles):
                    sts, idints = [], []
                    # group-major XT: xtb[p, g*D + c*128 + t] = x[slot g*128+t,
                    # d=c*128+p] so each group's xbar-transpose destination is
                    # contiguous (non-contiguous xbar dests are broken on HW)
                    xtb = pxt.tile([128, (TT // 128) * D], dt.bfloat16, tag="xtb")
                    xtb_v = xtb[:].rearrange("p (g c t) -> p g c t",
                                             g=TT // 128, c=D // 128)
                    for g in range(TT // 128):
                        kk = j * (TT // 128) + g
                        st = pst.tile([128, ROWW], dt.float32, tag="st")
                        nc.sync.dma_start(
                            st[:], staging.ap()[kk * 128:(kk + 1) * 128, :])
                        idint = pst.tile([128, 1], dt.int32, tag="idint")
                        nc.vector.tensor_copy(idint[:], st[:, 0:1])
                        sts.append(st)
                        idints.append(idint)
                        # DMA x-bar transpose: [128t, 1024d] -> [128d, 8c, 128t]
                        nc.sync.dma_start_transpose(
                            xtb_v[:, g],
                            st[:, XOFF:ROWW].bitcast(dt.bfloat16))

                    # h = gelu(x @ w1 + b1)   -> hT [F-chunk part, TT]
                    xtb_c = xtb[:].rearrange("p (g ct) -> p g ct", g=TT // 128)
                    for f in range(F // 128):
                        ph = pdH.tile([128, TT], dt.float32, tag="ph")
                        for c in range(D // 128):
                            nc.tensor.matmul(
                                ph[:],
                                lhsT=w1bf[:, c * F + f * 128: c * F + (f + 1) * 128],
                                rhs=xtb_c[:, :, c * 128:(c + 1) * 128],
                                start=(c == 0), stop=(c == D // 128 - 1))
                        nc.scalar.activation(
                            hT[:, f * TT:(f + 1) * TT], ph[:], Act.Gelu,
                            bias=b1p[:, f:f + 1])

                    # y = w2.T-contract(hT) + b2, xbar-transposed to token rows,
                    # scaled by combine weight, scattered by token id
                    ybig = ptk.tile([128, (TT // 128) * D], dt.bfloat16, tag="ybig")
                    ybig_v = ybig[:].rearrange("p (g q) -> p g q", g=TT // 128)
                    for hh in range(2):
                        pys_ps = [pdY.tile([128, TT], dt.float32, tag=f"psy{dd}",
                                           name=f"psy_{j}_{hh}_{dd}")
                                  for dd in range(4)]
                        for f in range(F // 128):
                            w2s = pw2.tile([128, 512], dt.bfloat16, tag="w2s")
                            nc.sync.dma_start(
                                w2s[:],
                                w2bf_dram.ap()[f * 128:(f + 1) * 128,
                                               hh * 512:(hh + 1) * 512])
                            for dd in range(4):
                                nc.tensor.matmul(
                                    pys_ps[dd][:],
                                    lhsT=w2s[:, dd * 128:(dd + 1) * 128],
                                    rhs=hT[:, f * TT:(f + 1) * TT],
                                    start=(f == 0), stop=(f == F // 128 - 1))
                        for dd in range(4):
                            d_ = 4 * hh + dd
                            ytb = pyb.tile([128, TT], dt.bfloat16, tag="ytb")
                            nc.vector.tensor_scalar(
                                out=ytb[:], in0=pys_ps[dd][:],
                                scalar1=b2p[:, d_:d_ + 1], scalar2=None,
                                op0=Alu.add)
                            # xbar: [128d, TT slots] -> [128 s, 4 g, 128 d]
                            ytt = pyb.tile([128, TT], dt.bfloat16, tag="ytt")
                            ytt_v = ytt[:].rearrange("p (g q) -> p g q",
                                                     g=TT // 128)
                            nc.sync.dma_start_transpose(ytt_v[:], ytb[:])
                            nc.vector.tensor_copy(
                                ybig_v[:, :, d_ * 128:(d_ + 1) * 128], ytt_v[:])
                    for g in range(TT // 128):
                        kk = j * (TT // 128) + g
                        yg = ybig[:, g * D:(g + 1) * D]
                        nc.vector.tensor_scalar(
                            out=yg, in0=yg, scalar1=sts[g][:, 1:2],
                            scalar2=None, op0=Alu.mult)
                        if SPARSE:
                            nc.gpsimd.indirect_dma_start(
                                out=out_buf.ap(),
                                out_offset=bass.IndirectOffsetOnAxis(
                                    ap=idints[g][:, 0:1], axis=0),
                                in_=yg, in_offset=None,
                                bounds_check=T - 1, oob_is_err=False)
                        else:
                            nc.sync.dma_start(
                                out_buf.ap()[kk * 128:(kk + 1) * 128, :], yg)

            # ---- phase R: combine across cores + emit shard ----
            if SKIP_RS:
                nc.sync.dma_start(rs_out.ap(), out_buf.ap()[:T // NCORES, :])
            else:
                nc.gpsimd.collective_compute(
                    "ReduceScatter",
                    mybir.AluOpType.add,
                    replica_groups=[list(range(NCORES))],
                    ins=[out_buf.ap().opt()],
                    outs=[rs_out.ap().opt()],
                )
            with tc.tile_pool(name="po_sb", bufs=2) as po:
                for k in range(T // NCORES // 128):
                    ob = po.tile([128, D], dt.bfloat16, tag="ob")
                    nc.sync.dma_start(ob[:], rs_out.ap()[k * 128:(k + 1) * 128, :])
                    of = po.tile([128, D], dt.float32, tag="of")
                    nc.scalar.copy(of[:], ob[:])
                    nc.sync.dma_start(out_ext.ap()[k * 128:(k + 1) * 128, :], of[:])

    nc.compile()
    return nc


def _make_inputs(inputs):
    hidden = np.ascontiguousarray(
        np.asarray(inputs["hidden_states"], dtype=np.float32).reshape(T, D))
    router_w = np.asarray(inputs["router_w"], dtype=np.float32)
    w1 = np.asarray(inputs["w1"], dtype=np.float32)
    b1 = np.asarray(inputs["b1"], dtype=np.float32)
    w2 = np.asarray(inputs["w2"], dtype=np.float32)
    b2 = np.asarray(inputs["b2"], dtype=np.float32)

    # packed router weights: rw[p, 8c+j] = router_w[128c+p, j]
    rw = np.ascontiguousarray(
        router_w.reshape(8, 128, E).transpose(1, 0, 2).reshape(128, 64))
    ids = np.ascontiguousarray(
        (np.arange(T, dtype=np.float32).reshape(NT, 128).T))
    tri = np.triu(np.ones((128, 128), dtype=np.float32), k=1)
    ones = np.ones((128, 128), dtype=np.float32)
    idf = np.eye(128, dtype=np.float32)

    in_maps = []
    for e in range(NCORES):
        esel = np.zeros((128, E), dtype=np.float32)
        esel[:, e] = 1.0
        b1p = np.ascontiguousarray(
            b1[e].reshape(F // 128, 128).T).astype(np.float32)
        b2p = np.ascontiguousarray(
            b2[e].reshape(D // 128, 128).T).astype(np.float32)
        in_maps.append({
            "hidden": hidden,
            "rw": rw,
            "w1": np.ascontiguousarray(w1[e]),
            "b1p": b1p,
            "w2": np.ascontiguousarray(w2[e]),
            "b2p": b2p,
            "esel": esel,
            "ids": ids,
            "tri": tri,
            "ones": ones,
            "idf": idf,
        })
    return in_maps


def kernel(**inputs):
    from concourse.bass_utils import run_bass_kernel_spmd

    if "nc" not in _STATE:
        _STATE["nc"] = _build()
    nc = _STATE["nc"]

    in_maps = _make_inputs(inputs)
    want_trace = os.environ.get("MOE_TRACE", "0") == "1"
    try:
        res = run_bass_kernel_spmd(nc, in_maps, list(range(NCORES)),
                                   trace=want_trace)
    except ModuleNotFoundError:
        res = run_bass_kernel_spmd(nc, in_maps, list(range(NCORES)),
                                   trace=False)
    _STATE["last_results"] = res

    out = np.concatenate(
        [np.asarray(res.results[c]["out"]) for c in range(NCORES)], axis=0)
    out = out.reshape(B, S, D).astype(np.float32)
    lb = np.float32(np.asarray(res.results[0]["lb"])[0, 0])
    return out, lb


# revision 23
# speedup vs baseline: 59.6064x; 1.0179x over previous
"""MoE layer (8 experts, top-2) on 8 TRN2 NeuronCores.

Expert-parallel with sparse token dispatch:
  - Every core receives the full hidden_states plus ONE expert's weights
    (host-sharded across cores) and replicated router weights.
  - Phase A (per core): stream token tiles; fp32 router matmuls (via PE
    transposes); exp/top-2 combine weights; a running matmul prefix-sum
    turns this expert's selection mask into compacted slot positions; a
    packed row [token_id, combine_w, pad, x(bf16)] is scattered into a
    capacity-bounded staging buffer by indirect DMA (OOB slots dropped).
  - Phase D: the expert MLP gelu(x@w1+b1)@w2+b2 runs over CAP staged
    slots in bf16 (fp32 accumulate); X and Y are transposed with the DMA
    x-bar; outputs are scaled by the combine weight and scattered back to
    a zeroed [8192,1024] bf16 buffer by token id.
  - ReduceScatter(+) over the 8 cores combines per-expert partials; each
    core emits its 1024-token shard; host concatenates.
  - Load-balance loss from softmax-prob partial sums (identical on every
    core).
"""
import os

import numpy as np

# ---- problem constants (hardcoded per contest contract) ----
B, S, D, F, E = 4, 2048, 1024, 4096, 8
T = B * S                    # 8192 tokens
NT = T // 128                # 64 token tiles
NCORES = 8
Z_LOSS_COEF = 0.001

SPARSE = os.environ.get("MOE_SPARSE", "1") == "1"
SKIP_RS = os.environ.get("MOE_SKIP_RS", "0") == "1"       # debug: no collective
SKIP_EXPERT = os.environ.get("MOE_SKIP_EXPERT", "0") == "1"  # debug
SKIP_ROUTER = os.environ.get("MOE_SKIP_ROUTER", "0") == "1"  # debug
CAP = int(os.environ.get("MOE_CAP", "2560"))   # per-expert token capacity
TT = 512                      # expert-phase token tile
XOFF = 16                     # f32 words before x payload (32B-align for xbar)
ROWW = XOFF + D // 2          # packed staging row in f32 words
BIGPOS = 100000.0

_STATE = {}


def _build():
    import concourse.bass as bass
    import concourse.bacc as bacc
    import concourse.mybir as mybir
    import concourse.tile as tile

    dt = mybir.dt
    Act = mybir.ActivationFunctionType
    Alu = mybir.AluOpType

    nc = bacc.Bacc("TRN2", target_bir_lowering=False, debug=False,
                   num_devices=NCORES)

    # ---- I/O ----
    hidden = nc.dram_tensor("hidden", [T, D], dt.float32, kind="ExternalInput")
    rw_in = nc.dram_tensor("rw", [128, 64], dt.float32, kind="ExternalInput")
    w1_in = nc.dram_tensor("w1", [D, F], dt.float32, kind="ExternalInput")
    b1_in = nc.dram_tensor("b1p", [128, F // 128], dt.float32, kind="ExternalInput")
    w2_in = nc.dram_tensor("w2", [F, D], dt.float32, kind="ExternalInput")
    b2_in = nc.dram_tensor("b2p", [128, D // 128], dt.float32, kind="ExternalInput")
    esel_in = nc.dram_tensor("esel", [128, E], dt.float32, kind="ExternalInput")
    ids_in = nc.dram_tensor("ids", [128, NT], dt.float32, kind="ExternalInput")
    tri_in = nc.dram_tensor("tri", [128, 128], dt.float32, kind="ExternalInput")
    ones_in = nc.dram_tensor("ones", [128, 128], dt.float32, kind="ExternalInput")
    idf_in = nc.dram_tensor("idf", [128, 128], dt.float32, kind="ExternalInput")

    out_ext = nc.dram_tensor("out", [T // NCORES, D], dt.float32, kind="ExternalOutput")
    lb_ext = nc.dram_tensor("lb", [1, 1], dt.float32, kind="ExternalOutput")

    # ---- internal DRAM ----
    w2bf_dram = nc.dram_tensor("w2bf_dram", [F, D], dt.bfloat16)
    out_buf = nc.dram_tensor("out_buf", [T, D], dt.bfloat16)
    rs_out = nc.dram_tensor("rs_out", [T // NCORES, D], dt.bfloat16)
    if SPARSE:
        staging = nc.dram_tensor("staging", [CAP, ROWW], dt.float32)
    else:
        staging = nc.dram_tensor("staging", [T, ROWW], dt.float32)

    n_exp_tiles = (CAP if SPARSE else T) // TT

    with tile.TileContext(nc) as tc:
        with tc.tile_pool(name="const", bufs=1) as cp:
            # ---- persistent tiles ----
            rw = cp.tile([128, 64], dt.float32, tag="rw")
            esel = cp.tile([128, E], dt.float32, tag="esel")
            ids = cp.tile([128, NT], dt.float32, tag="ids")
            tri = cp.tile([128, 128], dt.float32, tag="tri")
            ones = cp.tile([128, 128], dt.float32, tag="ones")
            idf = cp.tile([128, 128], dt.float32, tag="idf")
            b1p = cp.tile([128, F // 128], dt.float32, tag="b1p")
            b2p = cp.tile([128, D // 128], dt.float32, tag="b2p")
            for t_, src in ((rw, rw_in), (esel, esel_in), (ids, ids_in),
                            (tri, tri_in), (ones, ones_in), (idf, idf_in),
                            (b1p, b1_in), (b2p, b2_in)):
                nc.sync.dma_start(t_[:], src.ap())

            acc = cp.tile([128, E], dt.float32, tag="acc")
            nc.vector.memset(acc[:], 0.0)
            base = cp.tile([1, 1], dt.float32, tag="base")
            nc.vector.memset(base[:], 0.0)

            w1bf = cp.tile([128, D // 128 * F], dt.bfloat16, tag="w1bf")
            hT = cp.tile([128, F // 128 * TT], dt.bfloat16, tag="hT")

            # zero-fill output scatter buffer + pad-fill staging
            zrow = cp.tile([128, D], dt.bfloat16, tag="zrow")
            nc.vector.memset(zrow[:], 0.0)
            if SPARSE:
                for k in range(NT):
                    nc.sync.dma_start(out_buf.ap()[k * 128:(k + 1) * 128, :], zrow[:])
                padrow = cp.tile([128, ROWW], dt.float32, tag="padrow")
                nc.vector.memset(padrow[:], 0.0)
                nc.vector.memset(padrow[:, 0:1], float(T))
                for g in range(CAP // 128):
                    nc.sync.dma_start(
                        staging.ap()[g * 128:(g + 1) * 128, :], padrow[:])

            # ---- phases W (weight conversion) + A (router/dispatch) ----
            # co-allocated pools so the scheduler overlaps the two phases
            with (
                tc.tile_pool(name="wconv", bufs=2) as wp,
                tc.tile_pool(name="pa_sb", bufs=3) as pa,
                tc.tile_pool(name="pa_pk", bufs=3) as ppk,
                tc.tile_pool(name="pa_small", bufs=4) as pas,
                tc.tile_pool(name="pa_psT", bufs=2, space="PSUM") as paT,
                tc.tile_pool(name="pa_psL", bufs=2, space="PSUM") as paL,
                tc.tile_pool(name="pa_psP", bufs=1, space="PSUM") as paP,
            ):
                for c in range(D // 128):
                    wst = wp.tile([128, F], dt.float32, tag="w1st")
                    nc.sync.dma_start(wst[:], w1_in.ap()[c * 128:(c + 1) * 128, :])
                    nc.scalar.copy(w1bf[:, c * F:(c + 1) * F], wst[:])
                for c in range(F // 128):
                    st2 = wp.tile([128, D], dt.float32, tag="w2st")
                    nc.sync.dma_start(st2[:], w2_in.ap()[c * 128:(c + 1) * 128, :])
                    st2b = wp.tile([128, D], dt.bfloat16, tag="w2stb")
                    nc.vector.tensor_copy(st2b[:], st2[:])
                    nc.sync.dma_start(w2bf_dram.ap()[c * 128:(c + 1) * 128, :], st2b[:])

                for k in range(0 if SKIP_ROUTER else NT):
                    x = pa.tile([128, D], dt.float32, tag="x")
                    nc.sync.dma_start(x[:], hidden.ap()[k * 128:(k + 1) * 128, :])

                    xt = pa.tile([128, D], dt.float32, tag="xt")
                    for h in range(2):
                        pt = paT.tile([128, 512], dt.float32, tag="pt")
                        for c4 in range(4):
                            c = 4 * h + c4
                            nc.tensor.transpose(
                                pt[:, c4 * 128:(c4 + 1) * 128],
                                x[:, c * 128:(c + 1) * 128], idf[:])
                        nc.vector.tensor_copy(xt[:, h * 512:(h + 1) * 512], pt[:])

                    pl = paL.tile([128, E], dt.float32, tag="pl")
                    for c in range(8):
                        nc.tensor.matmul(
                            pl[:], lhsT=xt[:, c * 128:(c + 1) * 128],
                            rhs=rw[:, c * 8:(c + 1) * 8],
                            start=(c == 0), stop=(c == 7))

                    ex = pas.tile([128, E], dt.float32, tag="ex")
                    ssum = pas.tile([128, 1], dt.float32, tag="ssum")
                    nc.scalar.activation(ex[:], pl[:], Act.Exp, accum_out=ssum[:])
                    srec = pas.tile([128, 1], dt.float32, tag="srec")
                    nc.vector.reciprocal(srec[:], ssum[:])
                    m8 = pas.tile([128, 8], dt.float32, tag="m8")
                    nc.vector.max(m8[:], ex[:])
                    dn = pas.tile([128, 1], dt.float32, tag="dn")
                    nc.vector.tensor_tensor(
                        out=dn[:], in0=m8[:, 0:1], in1=m8[:, 1:2], op=Alu.add)
                    rd = pas.tile([128, 1], dt.float32, tag="rd")
                    nc.vector.reciprocal(rd[:], dn[:])
                    cu = pas.tile([128, E], dt.float32, tag="cu")
                    nc.vector.scalar_tensor_tensor(
                        out=cu[:], in0=ex[:], scalar=m8[:, 1:2], in1=ex[:],
                        op0=Alu.is_ge, op1=Alu.mult)
                    # packed row: [id, c, pad..., x bf16]
                    pk = ppk.tile([128, ROWW], dt.float32, tag="pk")
                    nc.vector.memset(pk[:, 2:XOFF], 0.0)
                    scr = pas.tile([128, E], dt.float32, tag="scr")
                    nc.vector.scalar_tensor_tensor(
                        out=scr[:], in0=cu[:], scalar=rd[:], in1=esel[:],
                        op0=Alu.mult, op1=Alu.mult,
                        accum_out=pk[:, 1:2])
                    nc.vector.scalar_tensor_tensor(
                        out=acc[:], in0=ex[:], scalar=srec[:], in1=acc[:],
                        op0=Alu.mult, op1=Alu.add)

                    if SPARSE:
                        mask = pas.tile([128, 1], dt.float32, tag="mask")
                        nc.vector.tensor_scalar(
                            out=mask[:], in0=pk[:, 1:2], scalar1=0.0,
                            scalar2=None, op0=Alu.is_gt)
                        # slot = (# selected before this token) + running base
                        pos_ps = paP.tile([128, 1], dt.float32, tag="pos_ps")
                        nc.tensor.matmul(pos_ps[:], lhsT=tri[:], rhs=mask[:],
                                         start=True, stop=False)
                        nc.tensor.matmul(pos_ps[:], lhsT=ones[0:1, :],
                                         rhs=base[:], start=False, stop=True)
                        cs_ps = paP.tile([1, 1], dt.float32, tag="cs_ps")
                        nc.tensor.matmul(cs_ps[:], lhsT=mask[:],
                                         rhs=ones[:, 0:1], start=True, stop=True)
                        nc.vector.tensor_tensor(
                            out=base[:], in0=base[:], in1=cs_ps[:], op=Alu.add)
                        posf = pas.tile([128, 1], dt.float32, tag="posf")
                        nc.vector.scalar_tensor_tensor(
                            out=posf[:], in0=mask[:], scalar=-BIGPOS,
                            in1=pos_ps[:], op0=Alu.mult, op1=Alu.add)
                        nc.vector.tensor_scalar(
                            out=posf[:], in0=posf[:], scalar1=BIGPOS,
                            scalar2=None, op0=Alu.add)
                        posi = pas.tile([128, 1], dt.int32, tag="posi")
                        nc.vector.tensor_copy(posi[:], posf[:])

                        nc.vector.tensor_copy(pk[:, 0:1], ids[:, k:k + 1])
                        nc.vector.tensor_copy(
                            pk[:, XOFF:ROWW].bitcast(dt.bfloat16), x[:])
                        nc.gpsimd.indirect_dma_start(
                            out=staging.ap(),
                            out_offset=bass.IndirectOffsetOnAxis(
                                ap=posi[:, 0:1], axis=0),
                            in_=pk[:], in_offset=None,
                            bounds_check=CAP - 1, oob_is_err=False)
                    else:
                        nc.vector.tensor_copy(pk[:, 0:1], ids[:, k:k + 1])
                        nc.vector.tensor_copy(
                            pk[:, XOFF:ROWW].bitcast(dt.bfloat16), x[:])
                        nc.sync.dma_start(
                            staging.ap()[k * 128:(k + 1) * 128, :], pk[:])

            # ---- load-balance loss ----
            with (
                tc.tile_pool(name="lb_sb", bufs=1) as lp,
                tc.tile_pool(name="lb_ps", bufs=1, space="PSUM") as lpp,
            ):
                loadr = lp.tile([1, E], dt.float32, tag="loadr")
                lps = lpp.tile([1, E], dt.float32, tag="lps")
                nc.tensor.matmul(lps[:], lhsT=ones[:, 0:1], rhs=acc[:],
                                 start=True, stop=True)
                nc.vector.tensor_scalar(
                    out=loadr[:], in0=lps[:], scalar1=1.0 / T,
                    scalar2=-1.0 / E, op0=Alu.mult, op1=Alu.add)
                sq = lp.tile([1, E], dt.float32, tag="sq")
                nc.vector.tensor_tensor(out=sq[:], in0=loadr[:], in1=loadr[:],
                                        op=Alu.mult)
                lbv = lp.tile([1, 1], dt.float32, tag="lbv")
                nc.vector.tensor_reduce(
                    out=lbv[:], in_=sq[:], axis=mybir.AxisListType.X, op=Alu.add)
                nc.vector.tensor_scalar(
                    out=lbv[:], in0=lbv[:], scalar1=Z_LOSS_COEF / E,
                    scalar2=None, op0=Alu.mult)
                nc.sync.dma_start(lb_ext.ap(), lbv[:])

            # ---- phase D: expert MLP over staged tokens ----
            with (
                tc.tile_pool(name="pd_st", bufs=8) as pst,
                tc.tile_pool(name="pd_xt", bufs=2) as pxt,
                tc.tile_pool(name="pd_w2", bufs=4) as pw2,
                tc.tile_pool(name="pd_yb", bufs=4) as pyb,
                tc.tile_pool(name="pd_tok", bufs=2) as ptk,
                tc.tile_pool(name="pd_psH", bufs=2, space="PSUM") as pdH,
                tc.tile_pool(name="pd_psY", bufs=1, space="PSUM") as pdY,
            ):
                for j in range(0 if SKIP_EXPERT else n_exp_tiles):
                    sts, idints = [], []
                    # group-major XT: xtb[p, g*D + c*128 + t] = x[slot g*128+t,
                    # d=c*128+p] so each group's xbar-transpose destination is
                    # contiguous (non-contiguous xbar dests are broken on HW)
                    xtb = pxt.tile([128, (TT // 128) * D], dt.bfloat16, tag="xtb")
                    xtb_v = xtb[:].rearrange("p (g c t) -> p g c t",
                                             g=TT // 128, c=D // 128)
                    for g in range(TT // 128):
                        kk = j * (TT // 128) + g
                        st = pst.tile([128, ROWW], dt.float32, tag="st")
                        nc.scalar.dma_start(
                            st[:], staging.ap()[kk * 128:(kk + 1) * 128, :])
                        idint = pst.tile([128, 1], dt.int32, tag="idint")
                        nc.vector.tensor_copy(idint[:], st[:, 0:1])
                        sts.append(st)
                        idints.append(idint)
                        # DMA x-bar transpose: [128t, 1024d] -> [128d, 8c, 128t]
                        nc.sync.dma_start_transpose(
                            xtb_v[:, g],
                            st[:, XOFF:ROWW].bitcast(dt.bfloat16))

                    # h = gelu(x @ w1 + b1)   -> hT [F-chunk part, TT]
                    xtb_c = xtb[:].rearrange("p (g ct) -> p g ct", g=TT // 128)
                    for f in range(F // 128):
                        ph = pdH.tile([128, TT], dt.float32, tag="ph")
                        for c in range(D // 128):
                            nc.tensor.matmul(
                                ph[:],
                                lhsT=w1bf[:, c * F + f * 128: c * F + (f + 1) * 128],
                                rhs=xtb_c[:, :, c * 128:(c + 1) * 128],
                                start=(c == 0), stop=(c == D // 128 - 1))
                        nc.scalar.activation(
                            hT[:, f * TT:(f + 1) * TT], ph[:], Act.Gelu,
                            bias=b1p[:, f:f + 1])

                    # y = w2.T-contract(hT) + b2, xbar-transposed to token rows,
                    # scaled by combine weight, scattered by token id
                    ybig = ptk.tile([128, (TT // 128) * D], dt.bfloat16, tag="ybig")
                    ybig_v = ybig[:].rearrange("p (g q) -> p g q", g=TT // 128)
                    for hh in range(2):
                        pys_ps = [pdY.tile([128, TT], dt.float32, tag=f"psy{dd}",
                                           name=f"psy_{j}_{hh}_{dd}")
                                  for dd in range(4)]
                        for f in range(F // 128):
                            w2s = pw2.tile([128, 512], dt.bfloat16, tag="w2s")
                            nc.scalar.dma_start(
                                w2s[:],
                                w2bf_dram.ap()[f * 128:(f + 1) * 128,
                                               hh * 512:(hh + 1) * 512])
                            for dd in range(4):
                                nc.tensor.matmul(
                                    pys_ps[dd][:],
                                    lhsT=w2s[:, dd * 128:(dd + 1) * 128],
                                    rhs=hT[:, f * TT:(f + 1) * TT],
                                    start=(f == 0), stop=(f == F // 128 - 1))
                        for dd in range(4):
                            d_ = 4 * hh + dd
                            ytb = pyb.tile([128, TT], dt.bfloat16, tag="ytb")
                            nc.vector.tensor_scalar(
                                out=ytb[:], in0=pys_ps[dd][:],
                                scalar1=b2p[:, d_:d_ + 1], scalar2=None,
                                op0=Alu.add)
                            # xbar: [128d, TT slots] -> [128 s, 4 g, 128 d]
                            ytt = pyb.tile([128, TT], dt.bfloat16, tag="ytt")
                            ytt_v = ytt[:].rearrange("p (g q) -> p g q",
                                                     g=TT // 128)
                            nc.sync.dma_start_transpose(ytt_v[:], ytb[:])
                            nc.vector.tensor_copy(
                                ybig_v[:, :, d_ * 128:(d_ + 1) * 128], ytt_v[:])
                    for g in range(TT // 128):
                        kk = j * (TT // 128) + g
                        yg = ybig[:, g * D:(g + 1) * D]
                        nc.vector.tensor_scalar(
                            out=yg, in0=yg, scalar1=sts[g][:, 1:2],
                            scalar2=None, op0=Alu.mult)
                        if SPARSE:
                            nc.gpsimd.indirect_dma_start(
                                out=out_buf.ap(),
                                out_offset=bass.IndirectOffsetOnAxis(
                                    ap=idints[g][:, 0:1], axis=0),
                                in_=yg, in_offset=None,
                                bounds_check=T - 1, oob_is_err=False)
                        else:
                            nc.sync.dma_start(
                                out_buf.ap()[kk * 128:(kk + 1) * 128, :], yg)

            # ---- phase R: combine across cores + emit shard ----
            if SKIP_RS:
                nc.sync.dma_start(rs_out.ap(), out_buf.ap()[:T // NCORES, :])
            else:
                nc.gpsimd.collective_compute(
                    "ReduceScatter",
                    mybir.AluOpType.add,
                    replica_groups=[list(range(NCORES))],
                    ins=[out_buf.ap().opt()],
                    outs=[rs_out.ap().opt()],
                )
            with tc.tile_pool(name="po_sb", bufs=2) as po:
                for k in range(T // NCORES // 128):
                    ob = po.tile([128, D], dt.bfloat16, tag="ob")
                    nc.sync.dma_start(ob[:], rs_out.ap()[k * 128:(k + 1) * 128, :])
                    of = po.tile([128, D], dt.float32, tag="of")
                    nc.scalar.copy(of[:], ob[:])
                    nc.sync.dma_start(out_ext.ap()[k * 128:(k + 1) * 128, :], of[:])

    nc.compile()
    return nc


def _make_inputs(inputs):
    hidden = np.ascontiguousarray(
        np.asarray(inputs["hidden_states"], dtype=np.float32).reshape(T, D))
    router_w = np.asarray(inputs["router_w"], dtype=np.float32)
    w1 = np.asarray(inputs["w1"], dtype=np.float32)
    b1 = np.asarray(inputs["b1"], dtype=np.float32)
    w2 = np.asarray(inputs["w2"], dtype=np.float32)
    b2 = np.asarray(inputs["b2"], dtype=np.float32)

    # packed router weights: rw[p, 8c+j] = router_w[128c+p, j]
    rw = np.ascontiguousarray(
        router_w.reshape(8, 128, E).transpose(1, 0, 2).reshape(128, 64))
    ids = np.ascontiguousarray(
        (np.arange(T, dtype=np.float32).reshape(NT, 128).T))
    tri = np.triu(np.ones((128, 128), dtype=np.float32), k=1)
    ones = np.ones((128, 128), dtype=np.float32)
    idf = np.eye(128, dtype=np.float32)

    in_maps = []
    for e in range(NCORES):
        esel = np.zeros((128, E), dtype=np.float32)
        esel[:, e] = 1.0
        b1p = np.ascontiguousarray(
            b1[e].reshape(F // 128, 128).T).astype(np.float32)
        b2p = np.ascontiguousarray(
            b2[e].reshape(D // 128, 128).T).astype(np.float32)
        in_maps.append({
            "hidden": hidden,
            "rw": rw,
            "w1": np.ascontiguousarray(w1[e]),
            "b1p": b1p,
            "w2": np.ascontiguousarray(w2[e]),
            "b2p": b2p,
            "esel": esel,
            "ids": ids,
            "tri": tri,
            "ones": ones,
            "idf": idf,
        })
    return in_maps


def kernel(**inputs):
    from concourse.bass_utils import run_bass_kernel_spmd

    if "nc" not in _STATE:
        _STATE["nc"] = _build()
    nc = _STATE["nc"]

    in_maps = _make_inputs(inputs)
    want_trace = os.environ.get("MOE_TRACE", "0") == "1"
    try:
        res = run_bass_kernel_spmd(nc, in_maps, list(range(NCORES)),
                                   trace=want_trace)
    except ModuleNotFoundError:
        res = run_bass_kernel_spmd(nc, in_maps, list(range(NCORES)),
                                   trace=False)
    _STATE["last_results"] = res

    out = np.concatenate(
        [np.asarray(res.results[c]["out"]) for c in range(NCORES)], axis=0)
    out = out.reshape(B, S, D).astype(np.float32)
    lb = np.float32(np.asarray(res.results[0]["lb"])[0, 0])
    return out, lb


# revision 32
# speedup vs baseline: 63.2161x; 1.0606x over previous
"""MoE layer (8 experts, top-2) on 8 TRN2 NeuronCores.

Expert-parallel with sparse token dispatch:
  - Every core receives the full hidden_states plus ONE expert's weights
    (host-sharded across cores) and replicated router weights.
  - Phase A (per core): stream token tiles; fp32 router matmuls (via PE
    transposes); exp/top-2 combine weights; a running matmul prefix-sum
    turns this expert's selection mask into compacted slot positions; a
    packed row [token_id, combine_w, pad, x(bf16)] is scattered into a
    capacity-bounded staging buffer by indirect DMA (OOB slots dropped).
  - Phase D: the expert MLP gelu(x@w1+b1)@w2+b2 runs over CAP staged
    slots in bf16 (fp32 accumulate); X and Y are transposed with the DMA
    x-bar; outputs are scaled by the combine weight and scattered back to
    a zeroed [8192,1024] bf16 buffer by token id.
  - ReduceScatter(+) over the 8 cores combines per-expert partials; each
    core emits its 1024-token shard; host concatenates.
  - Load-balance loss from softmax-prob partial sums (identical on every
    core).
"""
import os

import numpy as np

# ---- problem constants (hardcoded per contest contract) ----
B, S, D, F, E = 4, 2048, 1024, 4096, 8
T = B * S                    # 8192 tokens
NT = T // 128                # 64 token tiles
NCORES = 8
Z_LOSS_COEF = 0.001

SPARSE = os.environ.get("MOE_SPARSE", "1") == "1"
SKIP_RS = os.environ.get("MOE_SKIP_RS", "0") == "1"       # debug: no collective
SKIP_EXPERT = os.environ.get("MOE_SKIP_EXPERT", "0") == "1"  # debug
SKIP_ROUTER = os.environ.get("MOE_SKIP_ROUTER", "0") == "1"  # debug
CAP = int(os.environ.get("MOE_CAP", "2560"))   # per-expert token capacity
TT = 512                      # expert-phase token tile
XOFF = 16                     # f32 words before x payload (32B-align for xbar)
ROWW = XOFF + D // 2          # packed staging row in f32 words
BIGPOS = 100000.0

_STATE = {}


def _build():
    import concourse.bass as bass
    import concourse.bacc as bacc
    import concourse.mybir as mybir
    import concourse.tile as tile

    dt = mybir.dt
    Act = mybir.ActivationFunctionType
    Alu = mybir.AluOpType

    nc = bacc.Bacc("TRN2", target_bir_lowering=False, debug=False,
                   num_devices=NCORES)

    # ---- I/O ----
    hidden = nc.dram_tensor("hidden", [T, D], dt.float32, kind="ExternalInput")
    rw_in = nc.dram_tensor("rw", [128, 64], dt.float32, kind="ExternalInput")
    w1_in = nc.dram_tensor("w1", [D, F], dt.float32, kind="ExternalInput")
    b1_in = nc.dram_tensor("b1p", [128, F // 128], dt.float32, kind="ExternalInput")
    w2_in = nc.dram_tensor("w2", [F, D], dt.float32, kind="ExternalInput")
    b2_in = nc.dram_tensor("b2p", [128, D // 128], dt.float32, kind="ExternalInput")
    esel_in = nc.dram_tensor("esel", [128, E], dt.float32, kind="ExternalInput")
    ids_in = nc.dram_tensor("ids", [128, NT], dt.float32, kind="ExternalInput")
    tri_in = nc.dram_tensor("tri", [128, 128], dt.float32, kind="ExternalInput")
    ones_in = nc.dram_tensor("ones", [128, 128], dt.float32, kind="ExternalInput")
    idf_in = nc.dram_tensor("idf", [128, 128], dt.float32, kind="ExternalInput")

    out_ext = nc.dram_tensor("out", [T // NCORES, D], dt.float32, kind="ExternalOutput")
    lb_ext = nc.dram_tensor("lb", [1, 1], dt.float32, kind="ExternalOutput")

    # ---- internal DRAM ----
    w2bf_dram = nc.dram_tensor("w2bf_dram", [F, D], dt.bfloat16)
    out_buf = nc.dram_tensor("out_buf", [T, D], dt.bfloat16)
    rs_out = nc.dram_tensor("rs_out", [T // NCORES, D], dt.bfloat16)
    if SPARSE:
        staging = nc.dram_tensor("staging", [CAP, ROWW], dt.float32)
    else:
        staging = nc.dram_tensor("staging", [T, ROWW], dt.float32)

    n_exp_tiles = (CAP if SPARSE else T) // TT

    with tile.TileContext(nc) as tc:
        with tc.tile_pool(name="const", bufs=1) as cp:
            # ---- persistent tiles ----
            rw = cp.tile([128, 64], dt.float32, tag="rw")
            esel = cp.tile([128, E], dt.float32, tag="esel")
            ids = cp.tile([128, NT], dt.float32, tag="ids")
            tri = cp.tile([128, 128], dt.float32, tag="tri")
            ones = cp.tile([128, 128], dt.float32, tag="ones")
            idf = cp.tile([128, 128], dt.float32, tag="idf")
            b1p = cp.tile([128, F // 128], dt.float32, tag="b1p")
            b2p = cp.tile([128, D // 128], dt.float32, tag="b2p")
            for t_, src in ((rw, rw_in), (esel, esel_in), (ids, ids_in),
                            (tri, tri_in), (ones, ones_in), (idf, idf_in),
                            (b1p, b1_in), (b2p, b2_in)):
                nc.sync.dma_start(t_[:], src.ap())

            acc = cp.tile([128, E], dt.float32, tag="acc")
            nc.vector.memset(acc[:], 0.0)
            base = cp.tile([1, 1], dt.float32, tag="base")
            nc.vector.memset(base[:], 0.0)

            w1bf = cp.tile([128, D // 128 * F], dt.bfloat16, tag="w1bf")
            hT = cp.tile([128, F // 128 * TT], dt.bfloat16, tag="hT")

            # zero-fill output scatter buffer + pad-fill staging (scalar queue
            # so the sync queue leads with phase A's token loads)
            zrow = cp.tile([128, D], dt.bfloat16, tag="zrow")
            nc.vector.memset(zrow[:], 0.0)
            if SPARSE:
                padrow = cp.tile([128, ROWW], dt.float32, tag="padrow")
                nc.vector.memset(padrow[:], 0.0)
                nc.vector.memset(padrow[:, 0:1], float(T))
                for g in range(CAP // 128):
                    nc.scalar.dma_start(
                        staging.ap()[g * 128:(g + 1) * 128, :], padrow[:])
                for k in range(NT):
                    nc.scalar.dma_start(out_buf.ap()[k * 128:(k + 1) * 128, :],
                                        zrow[:])

            # ---- phases W (weight conversion) + A (router/dispatch) ----
            # co-allocated pools so the scheduler overlaps the two phases
            with (
                tc.tile_pool(name="wconv", bufs=2) as wp,
                tc.tile_pool(name="pa_sb", bufs=3) as pa,
                tc.tile_pool(name="pa_pk", bufs=3) as ppk,
                tc.tile_pool(name="pa_small", bufs=4) as pas,
                tc.tile_pool(name="pa_psT", bufs=2, space="PSUM") as paT,
                tc.tile_pool(name="pa_psL", bufs=2, space="PSUM") as paL,
                tc.tile_pool(name="pa_psP", bufs=1, space="PSUM") as paP,
            ):
                for k in range(0 if SKIP_ROUTER else NT):
                    x = pa.tile([128, D], dt.float32, tag="x")
                    nc.sync.dma_start(x[:], hidden.ap()[k * 128:(k + 1) * 128, :])

                    pk = ppk.tile([128, ROWW], dt.float32, tag="pk")
                    nc.vector.memset(pk[:, 2:XOFF], 0.0)
                    xbv = pk[:, XOFF:ROWW].bitcast(dt.bfloat16)
                    nc.vector.tensor_copy(xbv, x[:])

                    # fp32 router (top-2 selection needs ~1e-7 logit accuracy)
                    xt = pa.tile([128, D], dt.float32, tag="xt")
                    for h in range(2):
                        pt = paT.tile([128, 512], dt.float32, tag="pt")
                        for c4 in range(4):
                            c = 4 * h + c4
                            nc.tensor.transpose(
                                pt[:, c4 * 128:(c4 + 1) * 128],
                                x[:, c * 128:(c + 1) * 128], idf[:])
                        nc.vector.tensor_copy(xt[:, h * 512:(h + 1) * 512], pt[:])
                    pl = paL.tile([128, E], dt.float32, tag="pl")
                    for c in range(8):
                        nc.tensor.matmul(
                            pl[:], lhsT=xt[:, c * 128:(c + 1) * 128],
                            rhs=rw[:, c * 8:(c + 1) * 8],
                            start=(c == 0), stop=(c == 7))

                    ex = pas.tile([128, E], dt.float32, tag="ex")
                    ssum = pas.tile([128, 1], dt.float32, tag="ssum")
                    nc.scalar.activation(ex[:], pl[:], Act.Exp, accum_out=ssum[:])
                    srec = pas.tile([128, 1], dt.float32, tag="srec")
                    nc.vector.reciprocal(srec[:], ssum[:])
                    m8 = pas.tile([128, 8], dt.float32, tag="m8")
                    nc.vector.max(m8[:], ex[:])
                    dn = pas.tile([128, 1], dt.float32, tag="dn")
                    nc.vector.tensor_tensor(
                        out=dn[:], in0=m8[:, 0:1], in1=m8[:, 1:2], op=Alu.add)
                    rd = pas.tile([128, 1], dt.float32, tag="rd")
                    nc.vector.reciprocal(rd[:], dn[:])
                    cu = pas.tile([128, E], dt.float32, tag="cu")
                    nc.vector.scalar_tensor_tensor(
                        out=cu[:], in0=ex[:], scalar=m8[:, 1:2], in1=ex[:],
                        op0=Alu.is_ge, op1=Alu.mult)
                    scr = pas.tile([128, E], dt.float32, tag="scr")
                    nc.vector.scalar_tensor_tensor(
                        out=scr[:], in0=cu[:], scalar=rd[:], in1=esel[:],
                        op0=Alu.mult, op1=Alu.mult,
                        accum_out=pk[:, 1:2])
                    nc.vector.scalar_tensor_tensor(
                        out=acc[:], in0=ex[:], scalar=srec[:], in1=acc[:],
                        op0=Alu.mult, op1=Alu.add)

                    if SPARSE:
                        mask = pas.tile([128, 1], dt.float32, tag="mask")
                        nc.vector.tensor_scalar(
                            out=mask[:], in0=pk[:, 1:2], scalar1=0.0,
                            scalar2=None, op0=Alu.is_gt)
                        # slot = (# selected before this token) + running base
                        pos_ps = paP.tile([128, 1], dt.float32, tag="pos_ps")
                        nc.tensor.matmul(pos_ps[:], lhsT=tri[:], rhs=mask[:],
                                         start=True, stop=False)
                        nc.tensor.matmul(pos_ps[:], lhsT=ones[0:1, :],
                                         rhs=base[:], start=False, stop=True)
                        cs_ps = paP.tile([1, 1], dt.float32, tag="cs_ps")
                        nc.tensor.matmul(cs_ps[:], lhsT=mask[:],
                                         rhs=ones[:, 0:1], start=True, stop=True)
                        nc.vector.tensor_tensor(
                            out=base[:], in0=base[:], in1=cs_ps[:], op=Alu.add)
                        posf = pas.tile([128, 1], dt.float32, tag="posf")
                        nc.vector.scalar_tensor_tensor(
                            out=posf[:], in0=mask[:], scalar=-BIGPOS,
                            in1=pos_ps[:], op0=Alu.mult, op1=Alu.add)
                        nc.vector.tensor_scalar(
                            out=posf[:], in0=posf[:], scalar1=BIGPOS,
                            scalar2=None, op0=Alu.add)
                        posi = pas.tile([128, 1], dt.int32, tag="posi")
                        nc.vector.tensor_copy(posi[:], posf[:])

                        nc.vector.tensor_copy(pk[:, 0:1], ids[:, k:k + 1])
                        nc.gpsimd.indirect_dma_start(
                            out=staging.ap(),
                            out_offset=bass.IndirectOffsetOnAxis(
                                ap=posi[:, 0:1], axis=0),
                            in_=pk[:], in_offset=None,
                            bounds_check=CAP - 1, oob_is_err=False)
                    else:
                        nc.vector.tensor_copy(pk[:, 0:1], ids[:, k:k + 1])
                        nc.sync.dma_start(
                            staging.ap()[k * 128:(k + 1) * 128, :], pk[:])

                # ---- phase W emitted after A so A's DMAs lead the queues;
                # runs on the scalar HWDGE queue in parallel with A ----
                for c in range(D // 128):
                    wst = wp.tile([128, F], dt.float32, tag="w1st")
                    nc.scalar.dma_start(wst[:], w1_in.ap()[c * 128:(c + 1) * 128, :])
                    nc.scalar.copy(w1bf[:, c * F:(c + 1) * F], wst[:])
                for c in range(F // 128):
                    st2 = wp.tile([128, D], dt.float32, tag="w2st")
                    nc.scalar.dma_start(st2[:], w2_in.ap()[c * 128:(c + 1) * 128, :])
                    st2b = wp.tile([128, D], dt.bfloat16, tag="w2stb")
                    nc.vector.tensor_copy(st2b[:], st2[:])
                    nc.scalar.dma_start(
                        w2bf_dram.ap()[c * 128:(c + 1) * 128, :], st2b[:])

            # ---- load-balance loss ----
            with (
                tc.tile_pool(name="lb_sb", bufs=1) as lp,
                tc.tile_pool(name="lb_ps", bufs=1, space="PSUM") as lpp,
            ):
                loadr = lp.tile([1, E], dt.float32, tag="loadr")
                lps = lpp.tile([1, E], dt.float32, tag="lps")
                nc.tensor.matmul(lps[:], lhsT=ones[:, 0:1], rhs=acc[:],
                                 start=True, stop=True)
                nc.vector.tensor_scalar(
                    out=loadr[:], in0=lps[:], scalar1=1.0 / T,
                    scalar2=-1.0 / E, op0=Alu.mult, op1=Alu.add)
                sq = lp.tile([1, E], dt.float32, tag="sq")
                nc.vector.tensor_tensor(out=sq[:], in0=loadr[:], in1=loadr[:],
                                        op=Alu.mult)
                lbv = lp.tile([1, 1], dt.float32, tag="lbv")
                nc.vector.tensor_reduce(
                    out=lbv[:], in_=sq[:], axis=mybir.AxisListType.X, op=Alu.add)
                nc.vector.tensor_scalar(
                    out=lbv[:], in0=lbv[:], scalar1=Z_LOSS_COEF / E,
                    scalar2=None, op0=Alu.mult)
                nc.sync.dma_start(lb_ext.ap(), lbv[:])

            # ---- phase D: expert MLP over staged tokens ----
            with (
                tc.tile_pool(name="pd_st", bufs=8) as pst,
                tc.tile_pool(name="pd_xt", bufs=2) as pxt,
                tc.tile_pool(name="pd_w2", bufs=4) as pw2,
                tc.tile_pool(name="pd_yb", bufs=4) as pyb,
                tc.tile_pool(name="pd_tok", bufs=2) as ptk,
                tc.tile_pool(name="pd_psH", bufs=2, space="PSUM") as pdH,
                tc.tile_pool(name="pd_psY", bufs=1, space="PSUM") as pdY,
            ):
                for j in range(0 if SKIP_EXPERT else n_exp_tiles):
                    sts, idints = [], []
                    # group-major XT: xtb[p, g*D + c*128 + t] = x[slot g*128+t,
                    # d=c*128+p] so each group's xbar-transpose destination is
                    # contiguous (non-contiguous xbar dests are broken on HW)
                    xtb = pxt.tile([128, (TT // 128) * D], dt.bfloat16, tag="xtb")
                    xtb_v = xtb[:].rearrange("p (g c t) -> p g c t",
                                             g=TT // 128, c=D // 128)
                    for g in range(TT // 128):
                        kk = j * (TT // 128) + g
                        st = pst.tile([128, ROWW], dt.float32, tag="st")
                        nc.scalar.dma_start(
                            st[:], staging.ap()[kk * 128:(kk + 1) * 128, :])
                        idint = pst.tile([128, 1], dt.int32, tag="idint")
                        nc.vector.tensor_copy(idint[:], st[:, 0:1])
                        sts.append(st)
                        idints.append(idint)
                        # DMA x-bar transpose: [128t, 1024d] -> [128d, 8c, 128t]
                        nc.sync.dma_start_transpose(
                            xtb_v[:, g],
                            st[:, XOFF:ROWW].bitcast(dt.bfloat16))

                    # h = gelu(x @ w1 + b1)   -> hT [F-chunk part, TT]
                    xtb_c = xtb[:].rearrange("p (g ct) -> p g ct", g=TT // 128)
                    for f in range(F // 128):
                        ph = pdH.tile([128, TT], dt.float32, tag="ph")
                        for c in range(D // 128):
                            nc.tensor.matmul(
                                ph[:],
                                lhsT=w1bf[:, c * F + f * 128: c * F + (f + 1) * 128],
                                rhs=xtb_c[:, :, c * 128:(c + 1) * 128],
                                start=(c == 0), stop=(c == D // 128 - 1))
                        nc.scalar.activation(
                            hT[:, f * TT:(f + 1) * TT], ph[:], Act.Gelu,
                            bias=b1p[:, f:f + 1])

                    # y = w2.T-contract(hT) + b2, xbar-transposed to token rows,
                    # scaled by combine weight, scattered by token id
                    ybig = ptk.tile([128, (TT // 128) * D], dt.bfloat16, tag="ybig")
                    ybig_v = ybig[:].rearrange("p (g q) -> p g q", g=TT // 128)
                    for hh in range(2):
                        pys_ps = [pdY.tile([128, TT], dt.float32, tag=f"psy{dd}",
                                           name=f"psy_{j}_{hh}_{dd}")
                                  for dd in range(4)]
                        for f in range(F // 128):
                            w2s = pw2.tile([128, 512], dt.bfloat16, tag="w2s")
                            nc.scalar.dma_start(
                                w2s[:],
                                w2bf_dram.ap()[f * 128:(f + 1) * 128,
                                               hh * 512:(hh + 1) * 512])
                            for dd in range(4):
                                nc.tensor.matmul(
                                    pys_ps[dd][:],
                                    lhsT=w2s[:, dd * 128:(dd + 1) * 128],
                                    rhs=hT[:, f * TT:(f + 1) * TT],
                                    start=(f == 0), stop=(f == F // 128 - 1))
                        for dd in range(4):
                            d_ = 4 * hh + dd
                            ytb = pyb.tile([128, TT], dt.bfloat16, tag="ytb")
                            nc.vector.tensor_scalar(
                                out=ytb[:], in0=pys_ps[dd][:],
                                scalar1=b2p[:, d_:d_ + 1], scalar2=None,
                                op0=Alu.add)
                            # xbar: [128d, TT slots] -> [128 s, 4 g, 128 d]
                            ytt = pyb.tile([128, TT], dt.bfloat16, tag="ytt")
                            ytt_v = ytt[:].rearrange("p (g q) -> p g q",
                                                     g=TT // 128)
                            nc.sync.dma_start_transpose(ytt_v[:], ytb[:])
                            nc.vector.tensor_copy(
                                ybig_v[:, :, d_ * 128:(d_ + 1) * 128], ytt_v[:])
                    for g in range(TT // 128):
                        kk = j * (TT // 128) + g
                        yg = ybig[:, g * D:(g + 1) * D]
                        nc.vector.tensor_scalar(
                            out=yg, in0=yg, scalar1=sts[g][:, 1:2],
                            scalar2=None, op0=Alu.mult)
                        if SPARSE:
                            nc.gpsimd.indirect_dma_start(
                                out=out_buf.ap(),
                                out_offset=bass.IndirectOffsetOnAxis(
                                    ap=idints[g][:, 0:1], axis=0),
                                in_=yg, in_offset=None,
                                bounds_check=T - 1, oob_is_err=False)
                        else:
                            nc.sync.dma_start(
                                out_buf.ap()[kk * 128:(kk + 1) * 128, :], yg)

            # ---- phase R: combine across cores + emit shard ----
            if SKIP_RS:
                nc.sync.dma_start(rs_out.ap(), out_buf.ap()[:T // NCORES, :])
            else:
                nc.gpsimd.collective_compute(
                    "ReduceScatter",
                    mybir.AluOpType.add,
                    replica_groups=[list(range(NCORES))],
                    ins=[out_buf.ap().opt()],
                    outs=[rs_out.ap().opt()],
                )
            with tc.tile_pool(name="po_sb", bufs=2) as po:
                for k in range(T // NCORES // 128):
                    ob = po.tile([128, D], dt.bfloat16, tag="ob")
                    nc.sync.dma_start(ob[:], rs_out.ap()[k * 128:(k + 1) * 128, :])
                    of = po.tile([128, D], dt.float32, tag="of")
                    nc.scalar.copy(of[:], ob[:])
                    nc.sync.dma_start(out_ext.ap()[k * 128:(k + 1) * 128, :], of[:])

    nc.compile()
    return nc


def _make_inputs(inputs):
    hidden = np.ascontiguousarray(
        np.asarray(inputs["hidden_states"], dtype=np.float32).reshape(T, D))
    router_w = np.asarray(inputs["router_w"], dtype=np.float32)
    w1 = np.asarray(inputs["w1"], dtype=np.float32)
    b1 = np.asarray(inputs["b1"], dtype=np.float32)
    w2 = np.asarray(inputs["w2"], dtype=np.float32)
    b2 = np.asarray(inputs["b2"], dtype=np.float32)

    # packed router weights: rw[p, 8c+j] = router_w[128c+p, j]
    rw = np.ascontiguousarray(
        router_w.reshape(8, 128, E).transpose(1, 0, 2).reshape(128, 64))
    ids = np.ascontiguousarray(
        (np.arange(T, dtype=np.float32).reshape(NT, 128).T))
    tri = np.triu(np.ones((128, 128), dtype=np.float32), k=1)
    ones = np.ones((128, 128), dtype=np.float32)
    idf = np.eye(128, dtype=np.float32)

    in_maps = []
    for e in range(NCORES):
        esel = np.zeros((128, E), dtype=np.float32)
        esel[:, e] = 1.0
        b1p = np.ascontiguousarray(
            b1[e].reshape(F // 128, 128).T).astype(np.float32)
        b2p = np.ascontiguousarray(
            b2[e].reshape(D // 128, 128).T).astype(np.float32)
        in_maps.append({
            "hidden": hidden,
            "rw": rw,
            "w1": np.ascontiguousarray(w1[e]),
            "b1p": b1p,
            "w2": np.ascontiguousarray(w2[e]),
            "b2p": b2p,
            "esel": esel,
            "ids": ids,
            "tri": tri,
            "ones": ones,
            "idf": idf,
        })
    return in_maps


def kernel(**inputs):
    from concourse.bass_utils import run_bass_kernel_spmd

    if "nc" not in _STATE:
        _STATE["nc"] = _build()
    nc = _STATE["nc"]

    in_maps = _make_inputs(inputs)
    want_trace = os.environ.get("MOE_TRACE", "0") == "1"
    try:
        res = run_bass_kernel_spmd(nc, in_maps, list(range(NCORES)),
                                   trace=want_trace)
    except ModuleNotFoundError:
        res = run_bass_kernel_spmd(nc, in_maps, list(range(NCORES)),
                                   trace=False)
    _STATE["last_results"] = res

    out = np.concatenate(
        [np.asarray(res.results[c]["out"]) for c in range(NCORES)], axis=0)
    out = out.reshape(B, S, D).astype(np.float32)
    lb = np.float32(np.asarray(res.results[0]["lb"])[0, 0])
    return out, lb
